# revision 33
# baseline (speedup 1.0000x reference)
"""GPT forward pass on 8 Trainium2 NeuronCores — sequence-parallel (SP8).

Model: B=2, S=1024, D=1024, H=16 heads (hd=64), L=6 layers, V=50257,
tied embedding head.

Sharding: the 2048 tokens are split into 16 causal blocks of 128;
core c (group g=c//4 over batch, rank r=c%4) owns query blocks
A=r and B=7-r of batch g (256 tokens), which balances causal attention
work.  Every core holds the FULL weights (bf16); the only per-layer
communication is a bf16 AllGather of K then V (~0.5 MB each) within
each 4-core group.  The final LN output is AllGathered once before the
vocab-sharded tied-logit matmul.

Perf structure (v2):
- Deferred LayerNorm: projections contract x~ = x - mean directly
  (built with one vector op per strip); the 1/sigma scale is folded
  into the PSUM evictions.  This removes the serial LN-finish chain
  between FFN and the next layer's QKV projections, so the K/V
  AllGather triggers earlier.
- Attention holds all 16 heads' exp(scores) tiles so the serialized
  V AllGather hides behind phase-1 (scores+exp) work.
- Softmax normalization is batched: denominators are copied into one
  row, one reciprocal_approx_fast over [1, 4096], broadcast back via
  tiny ones-matmuls, and applied with 8 vector ops (instead of 32
  slow [1,128] reciprocals + gpsimd broadcasts).
- Logits are emitted in bf16 (halves the output DMA) and upcast on
  the host.
"""

import sys

sys.path.insert(0, "/opt/trn_rl_repo")

import contextlib

import numpy as np
import ml_dtypes

import concourse.bacc as bacc
import concourse.mybir as mybir
import concourse.tile as tile
from concourse.bass import ts
from concourse.bass_utils import run_bass_kernel_spmd

F32 = mybir.dt.float32
F32R = mybir.dt.float32r
BF16 = mybir.dt.bfloat16
AF = mybir.ActivationFunctionType
ALU = mybir.AluOpType
BF16NP = ml_dtypes.bfloat16

# Model dims
B, S, D, H, L, V = 2, 1024, 1024, 16, 6, 50257
HD = D // H            # 64
DFF = 4 * D            # 4096
N_CORES = 8
G = 4                  # group size (cores per batch element)
KD = D // 128          # 8 feature tiles
HC = H // 2            # 8 head-chunks (2 heads per 128 partitions)
TPC = 256              # tokens per core
BLK = 128              # token block
M1 = DFF // 128        # 32 w1 out strips
M2 = KD                # 8 w2 out strips
NSLOT = 12             # attention slots per head (4 A-side + 8 B-side)
VS = 12565             # vocab rows per group-rank (last: 12562)
VSP = 12800            # padded
NVB = VSP // 128       # 100 vocab blocks
T = 1024               # tokens per group (gathered)

KCOLS = HC * TPC            # 2048 k cols in kv contribution
VSEG = H * (HD + 1)         # 1040 v cols per token block (ones col incl.)
NFCOLS = KD * TPC           # 2048

# key block kb -> (rank, slot-within-rank)
RHO = [kb if kb < 4 else 7 - kb for kb in range(8)]
SIG = [0 if kb < 4 else 1 for kb in range(8)]

REPLICA_GROUPS = [[0, 1, 2, 3], [4, 5, 6, 7]]


def _f(name, l=None):
    return name if l is None else f"{name}{l}"


def build_program(debug_taps=False):
    nc = bacc.Bacc("TRN2", target_bir_lowering=False, debug=False,
                   enable_asserts=True, num_devices=N_CORES)

    inp = {}

    def dram_in(name, shape, dtype=BF16):
        inp[name] = nc.dram_tensor(name, shape, dtype, kind="ExternalInput").ap()
        return inp[name]

    dram_in("x0", [128, KD, TPC], F32R)
    dram_in("ones", [128, 1], F32R)
    dram_in("m01", [128, NSLOT, BLK], BF16)
    for l in range(L):
        dram_in(_f("wq", l), [128, KD, KD, 128])    # [p, m, kt, 128]
        dram_in(_f("wk", l), [128, KD, KD, 128])
        dram_in(_f("wv", l), [128, KD, D])          # plain k-fold (moving)
        dram_in(_f("wo", l), [128, KD, KD, 128])
        dram_in(_f("w1", l), [128, M1, KD, 128])
        dram_in(_f("w2", l), [128, M2, M1, 128])
        dram_in(_f("b1", l), [128, M1], F32)
        dram_in(_f("b2", l), [128, M2], F32)
        dram_in(_f("nks", l), [128, KD], F32)       # -colsum(wk), per strip
    dram_in("emb", [128, NVB, KD, 128])
    logits = nc.dram_tensor("logits", [VSP, T], BF16, kind="ExternalOutput").ap()

    taps = {}
    if debug_taps:
        for name in ["xa_0", "x_1", "nf"]:
            taps[name] = nc.dram_tensor("dbg_" + name, [128, KD, TPC], F32,
                                        kind="ExternalOutput").ap()

    with tile.TileContext(nc) as tc:
        _body(tc, inp, logits, taps)
    nc.compile()
    return nc


def _body(tc, inp, logits, taps):
    nc = tc.nc
    ctx = contextlib.ExitStack()
    with ctx:
        # --- SBUF pools ---
        singles = ctx.enter_context(tc.tile_pool(name="singles", bufs=1))
        xp = ctx.enter_context(tc.tile_pool(name="xp", bufs=1))        # 8K
        npool = ctx.enter_context(tc.tile_pool(name="npool", bufs=2))  # 8K
        tmp = ctx.enter_context(tc.tile_pool(name="tmp", bufs=3))      # 3K
        qp = ctx.enter_context(tc.tile_pool(name="qp", bufs=1))        # 4K
        stg = ctx.enter_context(tc.tile_pool(name="stg", bufs=1))      # 8.25K
        kvp = ctx.enter_context(tc.tile_pool(name="kvp", bufs=1))      # 33K
        op = ctx.enter_context(tc.tile_pool(name="op", bufs=1))        # 4K
        hp = ctx.enter_context(tc.tile_pool(name="hp", bufs=1))        # 16K
        wq_p = ctx.enter_context(tc.tile_pool(name="wq_p", bufs=8))    # 16K
        wv_p = ctx.enter_context(tc.tile_pool(name="wv_p", bufs=4))    # 4K
        w2_p = ctx.enter_context(tc.tile_pool(name="w2_p", bufs=3))    # 24K
        embp = ctx.enter_context(tc.tile_pool(name="embp", bufs=2))    # 4K
        ep = ctx.enter_context(tc.tile_pool(name="ep", bufs=16))       # 48K
        stat = ctx.enter_context(tc.tile_pool(name="stat", bufs=2))
        bcp = ctx.enter_context(tc.tile_pool(name="bcp", bufs=2))      # 2K
        lout = ctx.enter_context(tc.tile_pool(name="lout", bufs=2))    # 2K
        bias = ctx.enter_context(tc.tile_pool(name="bias", bufs=2))
        # --- PSUM pools (8 banks) ---
        ps_mm = ctx.enter_context(tc.tile_pool(name="ps_mm", bufs=4, space="PSUM"))
        ps_po = ctx.enter_context(tc.tile_pool(name="ps_po", bufs=2, space="PSUM"))
        ps_st = ctx.enter_context(tc.tile_pool(name="ps_st", bufs=1, space="PSUM"))
        # --- DRAM (collective bounce) ---
        dram = ctx.enter_context(tc.tile_pool(name="dram", bufs=2, space="DRAM"))

        # --- constants / persistent ---
        ones_t = singles.tile([128, 1], F32R)
        nc.sync.dma_start(out=ones_t[:], in_=inp["ones"][:])
        eps_t = singles.tile([1, 1], F32)
        nc.vector.memset(eps_t[:], 1e-5)
        m01t = singles.tile([128, NSLOT, BLK], BF16)
        nc.sync.dma_start(out=m01t[:], in_=inp["m01"][:])

        xt = xp.tile([128, KD, TPC], F32R, tag="x")
        nc.sync.dma_start(out=xt[:], in_=inp["x0"][:])

        def ln_alloc(nm):
            s1 = ps_st.tile([1, TPC], F32, tag="st1", name=f"s1{nm}")
            s2 = ps_st.tile([1, TPC], F32, tag="st2", name=f"s2{nm}")
            return s1, s2

        def ln_stats_k(src, s1, s2, k):
            nc.tensor.matmul(s1[:], ones_t[:], src[:, k, :],
                             start=(k == 0), stop=(k == KD - 1))
            sq = tmp.tile([128, TPC], F32R, tag="tmp", name=f"sq_{k}")
            nc.vector.tensor_tensor(
                out=sq[:], in0=src[:, k, :].bitcast(F32),
                in1=src[:, k, :].bitcast(F32), op=ALU.mult)
            nc.tensor.matmul(s2[:], ones_t[:], sq[:],
                             start=(k == 0), stop=(k == KD - 1))

        def mean_chain(s1, nm):
            """s1 -> (m [1,TPC] f32, mB_s [128,TPC] f32 in SBUF)."""
            m = stat.tile([1, TPC], F32, tag="sa", name=f"m{nm}")
            nc.vector.tensor_scalar_mul(m[:], s1[:], 1.0 / D)
            mB_s = bcp.tile([128, TPC], F32, tag="mb", name=f"mb{nm}")
            nc.gpsimd.partition_broadcast(mB_s[:], m[:])
            return m, mB_s

        def xtilde(mB_s, nm):
            """x~ = x - mean, cast to bf16 (one op per strip)."""
            xtl = npool.tile([128, KD, TPC], BF16, tag="n", name=f"xt{nm}")
            for k in range(KD):
                nc.vector.tensor_tensor(
                    out=xtl[:, k, :], in0=xt[:, k, :].bitcast(F32),
                    in1=mB_s[:], op=ALU.subtract)
            return xtl

        def rs_chain(s2, m, nm, want_col=False):
            """-> rs_bs [128,TPC] f32 in SBUF (and rs_col [128,2] if asked)."""
            msq = stat.tile([1, TPC], F32, tag="sb", name=f"msq{nm}")
            nc.vector.tensor_tensor(out=msq[:], in0=m[:].bitcast(F32),
                                    in1=m[:].bitcast(F32), op=ALU.mult)
            var = stat.tile([1, TPC], F32, tag="sb", name=f"var{nm}")
            nc.vector.scalar_tensor_tensor(
                out=var[:], in0=s2[:], scalar=1.0 / D, in1=msq[:],
                op0=ALU.mult, op1=ALU.subtract)
            var2 = stat.tile([1, TPC], F32R, tag="sb", name=f"var2{nm}")
            nc.vector.tensor_scalar_add(var2[:], var[:], 1e-5)
            vr = stat.tile([1, TPC], F32, tag="sb", name=f"vr{nm}")
            nc.vector.reciprocal_approx_fast(vr[:], var2[:].bitcast(F32))
            rs = stat.tile([1, TPC], F32, tag="sb", name=f"rs{nm}")
            nc.scalar.activation(rs[:], vr[:], AF.Sqrt)
            rs_bs = bcp.tile([128, TPC], F32, tag="rs", name=f"rsb{nm}")
            nc.gpsimd.partition_broadcast(rs_bs[:], rs[:])
            return rs_bs

        def k_stage_gather(l, xb, mB_s, rs_bs, nks_t):
            """K = (wk^T xb - ksum*m) * rs -> staging -> AllGather.

            The matmuls contract the raw (uncentered) bf16 residual xb,
            so they queue immediately behind the previous FFN with no
            LN-stats dependency; the mean/scale fixup rides the PSUM
            eviction.  Staging DMAs go out per strip."""
            stage = stg.tile([128, KCOLS], BF16, tag="stgk", name=f"stagek{l}")
            ag_in = dram.tile([128, KCOLS], BF16, tag="agik", name=f"agik{l}")
            ag_out = dram.tile([G, 128, KCOLS], BF16, tag="agok",
                               name=f"agok{l}")
            for mp in range(KD // 2):
                pk = ps_mm.tile([128, 2, TPC], F32, tag="mm",
                                name=f"pk{l}_{mp}")
                for j in range(2):
                    m = 2 * mp + j
                    wkt = wq_p.tile([128, KD, 128], BF16, tag="wqk",
                                    name=f"wk{l}_{m}")
                    nc.sync.dma_start(out=wkt[:], in_=inp[_f("wk", l)][:, m])
                    for k in range(KD):
                        nc.tensor.matmul(pk[:, j, :], wkt[:, k, :],
                                         xb[:, k, :],
                                         start=(k == 0), stop=(k == KD - 1))
                    t = tmp.tile([128, TPC], F32, tag="tmp",
                                 name=f"kt{l}_{m}")
                    nc.vector.scalar_tensor_tensor(
                        out=t[:], in0=mB_s[:], scalar=nks_t[:, m:m + 1],
                        in1=pk[:, j, :], op0=ALU.mult, op1=ALU.add)
                    nc.vector.tensor_tensor(
                        out=stage[:, ts(m, TPC)], in0=t[:],
                        in1=rs_bs[:], op=ALU.mult)
                    nc.sync.dma_start(out=ag_in[:, ts(m, TPC)],
                                      in_=stage[:, ts(m, TPC)])
            nc.gpsimd.collective_compute(
                "AllGather", ALU.bypass, replica_groups=REPLICA_GROUPS,
                ins=[ag_in.opt()], outs=[ag_out.opt()])
            return ag_out

        def v_stage_gather(l, n1v):
            """V (token-major, + ones cols) -> staging -> AllGather."""
            stage = stg.tile([128, 2 * VSEG], BF16, tag="stgv",
                             name=f"stagev{l}")
            ones_view = stage[:].rearrange(
                "p (b h c) -> p b h c", b=2, h=H)[:, :, :, HD:]
            nc.vector.memset(ones_view, 1.0)
            for hh in range(2):   # halves of the head dim (512 cols)
                pva = ps_po.tile([128, 512], F32, tag="po",
                                 name=f"pva{l}_{hh}")
                pvb = ps_po.tile([128, 512], F32, tag="po",
                                 name=f"pvb{l}_{hh}")
                for k in range(KD):
                    wvs = wv_p.tile([128, 512], BF16, tag="wv",
                                    name=f"wv{l}_{hh}_{k}")
                    nc.sync.dma_start(out=wvs[:],
                                      in_=inp[_f("wv", l)][:, k, ts(hh, 512)])
                    nc.tensor.matmul(pva[:], n1v[:, k, ts(0, BLK)], wvs[:],
                                     start=(k == 0), stop=(k == KD - 1))
                    nc.tensor.matmul(pvb[:], n1v[:, k, ts(1, BLK)], wvs[:],
                                     start=(k == 0), stop=(k == KD - 1))
                for blk, pv in ((0, pva), (1, pvb)):
                    for j in range(8):
                        h = hh * 8 + j
                        nc.vector.tensor_scalar_mul(
                            stage[:, blk * VSEG + h * (HD + 1):
                                  blk * VSEG + h * (HD + 1) + HD],
                            pv[:, ts(j, HD)], 1.0)
            ag_in = dram.tile([128, 2 * VSEG], BF16, tag="agiv",
                              name=f"agiv{l}")
            ag_out = dram.tile([G, 128, 2 * VSEG], BF16, tag="agov",
                               name=f"agov{l}")
            nc.sync.dma_start(out=ag_in[:], in_=stage[:])
            nc.gpsimd.collective_compute(
                "AllGather", ALU.bypass, replica_groups=REPLICA_GROUPS,
                ins=[ag_in.opt()], outs=[ag_out.opt()])
            return ag_out

        def q_proj(l, xtl, rs_bs):
            q = qp.tile([128, KD, TPC], BF16, tag="q", name=f"q{l}")
            for mp in range(KD // 2):
                pq = ps_mm.tile([128, 2, TPC], F32, tag="mm",
                                name=f"pq{l}_{mp}")
                for j in range(2):
                    m = 2 * mp + j
                    wqt = wq_p.tile([128, KD, 128], BF16, tag="wqk",
                                    name=f"wq{l}_{m}")
                    nc.sync.dma_start(out=wqt[:], in_=inp[_f("wq", l)][:, m])
                    for k in range(KD):
                        nc.tensor.matmul(pq[:, j, :], wqt[:, k, :],
                                         xtl[:, k, :],
                                         start=(k == 0), stop=(k == KD - 1))
                    nc.vector.tensor_tensor(
                        out=q[:, m, :], in0=pq[:, j, :], in1=rs_bs[:],
                        op=ALU.mult)
            return q

        def attention(l, q, kg, vg, oT):

            def phase1(h):
                pp = 64 * (h % 2)
                hc = h // 2
                eTile = ep.tile([128, NSLOT, BLK], BF16, tag="eT",
                                name=f"eT{l}_{h}")
                # scores (transposed [k, q]) + exp + causal mask,
                # batched 4 slots per PSUM bank
                for grp in range(NSLOT // 4):
                    pss = ps_mm.tile([128, 4, BLK], F32, tag="mm",
                                     name=f"sc{l}_{h}_{grp}")
                    for j in range(4):
                        s = 4 * grp + j
                        kb = s if s < 4 else s - 4
                        qc = ts(0, BLK) if s < 4 else ts(1, BLK)
                        rho, sg = RHO[kb], SIG[kb]
                        nc.tensor.matmul(
                            pss[:, j, :],
                            kg[pp:pp + 64, rho,
                               hc * TPC + sg * BLK: hc * TPC + sg * BLK + BLK],
                            q[pp:pp + 64, hc, qc], start=True, stop=True)
                    nc.scalar.activation(eTile[:, ts(grp, 4), :], pss[:],
                                         AF.Exp)
                    nc.vector.tensor_tensor(
                        out=eTile[:, ts(grp, 4), :],
                        in0=eTile[:, ts(grp, 4), :],
                        in1=m01t[:, ts(grp, 4), :], op=ALU.mult)
                return eTile

            def phase2(h, eTile):
                pp = 64 * (h % 2)
                hc = h // 2
                # AV (+ ones-column denominator); A-side (cols 0:128) and
                # B-side (cols 128:256) are two accumulation groups in ONE
                # bank — the B start=True clears only has_written bits, the
                # already-final A values persist.
                pso = ps_po.tile([HD + 1, TPC], F32, tag="po",
                                 name=f"pav{l}_{h}")
                for s in range(NSLOT):
                    kb = s if s < 4 else s - 4
                    rho, sg = RHO[kb], SIG[kb]
                    cc = ts(0, BLK) if s < 4 else ts(1, BLK)
                    vsl = vg[:, rho,
                             sg * VSEG + h * (HD + 1):
                             sg * VSEG + (h + 1) * (HD + 1)]
                    nc.tensor.matmul(pso[:, cc], vsl, eTile[:, s, :],
                                     start=(s in (0, 4)),
                                     stop=(s in (3, NSLOT - 1)),
                                     skip_group_check=True)
                r_ = stat.tile([1, TPC], F32, tag="rr", name=f"r{l}_{h}")
                nc.vector.reciprocal(r_[:], pso[HD:HD + 1, :])
                bb = bcp.tile([64, TPC], F32, tag="bb", name=f"bb{l}_{h}")
                nc.gpsimd.partition_broadcast(bb[:], r_[:])
                nc.vector.tensor_tensor(
                    out=oT[pp:pp + 64, hc, :], in0=pso[0:HD, :],
                    in1=bb[:], op=ALU.mult)

            held = [phase1(h) for h in range(H)]
            for h in range(H):
                phase2(h, held[h])

        def wo_residual(l, oT, s1, s2):
            for mp in range(KD // 2):
                po = ps_mm.tile([128, 2, TPC], F32, tag="mm",
                                name=f"po{l}_{mp}")
                for j in range(2):
                    m = 2 * mp + j
                    wot = wq_p.tile([128, KD, 128], BF16, tag="wqk",
                                    name=f"wo{l}_{m}")
                    nc.sync.dma_start(out=wot[:], in_=inp[_f("wo", l)][:, m])
                    for k in range(KD):
                        nc.tensor.matmul(po[:, j, :], wot[:, k, :],
                                         oT[:, k, :],
                                         start=(k == 0), stop=(k == KD - 1))
                    nc.vector.tensor_tensor(
                        out=xt[:, m, :], in0=xt[:, m, :].bitcast(F32),
                        in1=po[:, j, :], op=ALU.add)
                for j in range(2):
                    ln_stats_k(xt, s1, s2, 2 * mp + j)

        def ffn(l, xtl2, rs_bs2, s1, s2, xb_next):
            b1_t = bias.tile([128, M1], F32, tag="bias", name=f"b1{l}")
            nc.sync.dma_start(out=b1_t[:], in_=inp[_f("b1", l)][:])
            b2_t = bias.tile([128, M2], F32, tag="bias", name=f"b2{l}")
            nc.sync.dma_start(out=b2_t[:], in_=inp[_f("b2", l)][:])
            hT = hp.tile([128, M1, TPC], BF16, tag="h", name=f"hT{l}")
            for mp in range(M1 // 2):
                p1 = ps_mm.tile([128, 2, TPC], F32, tag="mm",
                                name=f"p1{l}_{mp}")
                for j in range(2):
                    m = 2 * mp + j
                    w1t = wq_p.tile([128, KD, 128], BF16, tag="wqk",
                                    name=f"w1{l}_{m}")
                    nc.sync.dma_start(out=w1t[:], in_=inp[_f("w1", l)][:, m])
                    for k in range(KD):
                        nc.tensor.matmul(p1[:, j, :], w1t[:, k, :],
                                         xtl2[:, k, :],
                                         start=(k == 0), stop=(k == KD - 1))
                    u = tmp.tile([128, TPC], F32, tag="tmp",
                                 name=f"u{l}_{m}")
                    nc.vector.tensor_tensor(
                        out=u[:], in0=p1[:, j, :], in1=rs_bs2[:],
                        op=ALU.mult)
                    nc.scalar.activation(hT[:, m, :], u[:], AF.Gelu,
                                         bias=b1_t[:, m:m + 1])
            for mp in range(M2 // 2):
                p2 = ps_mm.tile([128, 2, TPC], F32, tag="mm",
                                name=f"p2{l}_{mp}")
                for j in range(2):
                    m = 2 * mp + j
                    w2t = w2_p.tile([128, M1, 128], BF16, tag="w2",
                                    name=f"w2{l}_{m}")
                    nc.sync.dma_start(out=w2t[:], in_=inp[_f("w2", l)][:, m])
                    for k in range(M1):
                        nc.tensor.matmul(p2[:, j, :], w2t[:, k, :],
                                         hT[:, k, :],
                                         start=(k == 0), stop=(k == M1 - 1))
                    nc.vector.scalar_tensor_tensor(
                        out=xt[:, m, :], in0=p2[:, j, :],
                        scalar=b2_t[:, m:m + 1],
                        in1=xt[:, m, :].bitcast(F32), op0=ALU.add,
                        op1=ALU.add)
                    nc.scalar.copy(xb_next[:, m, :],
                                   xt[:, m, :].bitcast(F32))
                for j in range(2):
                    ln_stats_k(xt, s1, s2, 2 * mp + j)

        def tap_f32(name):
            if name in taps:
                nc.sync.dma_start(out=taps[name][:], in_=xt[:].bitcast(F32))

        # ---------------- main loop ----------------
        s1, s2 = ln_alloc("ln_0")
        xb = npool.tile([128, KD, TPC], BF16, tag="n", name="xb0")
        for k in range(KD):
            ln_stats_k(xt, s1, s2, k)
            nc.scalar.copy(xb[:, k, :], xt[:, k, :].bitcast(F32))
        for l in range(L):
            nks_t = bias.tile([128, KD], F32, tag="nks", name=f"nks{l}")
            nc.sync.dma_start(out=nks_t[:], in_=inp[_f("nks", l)][:])
            m, mB_s = mean_chain(s1, f"l{l}")
            rs_bs = rs_chain(s2, m, f"l{l}")
            agk = k_stage_gather(l, xb, mB_s, rs_bs, nks_t)
            xtl = xtilde(mB_s, f"l{l}")
            n1v = npool.tile([128, KD, TPC], BF16, tag="n", name=f"n1v{l}")
            for k in range(KD):
                nc.vector.tensor_tensor(
                    out=n1v[:, k, :], in0=xtl[:, k, :], in1=rs_bs[:],
                    op=ALU.mult)
            agv = v_stage_gather(l, n1v)
            q = q_proj(l, xtl, rs_bs)
            kg = kvp.tile([128, G, KCOLS], BF16, tag="kg", name=f"kg{l}")
            for rho in range(G):
                nc.sync.dma_start(out=kg[:, rho, :], in_=agk[rho])
            vg = kvp.tile([128, G, 2 * VSEG], BF16, tag="vg", name=f"vg{l}")
            for rho in range(G):
                nc.sync.dma_start(out=vg[:, rho, :], in_=agv[rho])
            oT = op.tile([128, KD, TPC], BF16, tag="oT", name=f"oT{l}")
            attention(l, q, kg, vg, oT)
            s1a, s2a = ln_alloc(f"ln2_{l}")
            wo_residual(l, oT, s1a, s2a)
            if l == 0:
                tap_f32("xa_0")
            m2, m2B_s = mean_chain(s1a, f"f{l}")
            xtl2 = xtilde(m2B_s, f"f{l}")
            rs_bs2 = rs_chain(s2a, m2, f"f{l}")
            s1, s2 = ln_alloc(f"ln1_{l + 1}")
            xb = npool.tile([128, KD, TPC], BF16, tag="n", name=f"xb{l + 1}")
            ffn(l, xtl2, rs_bs2, s1, s2, xb)
            if l == 0:
                tap_f32("x_1")

        # final LN -> nf
        mf, mfB_s = mean_chain(s1, "fin")
        xtlf = xtilde(mfB_s, "fin")
        rs_bsf = rs_chain(s2, mf, "fin")
        nf = npool.tile([128, KD, TPC], BF16, tag="n", name="nf")
        for k in range(KD):
            nc.vector.tensor_tensor(
                out=nf[:, k, :], in0=xtlf[:, k, :], in1=rs_bsf[:],
                op=ALU.mult)
        if "nf" in taps:
            f = stg.tile([128, KD, TPC], F32, tag="tapf", name="tpnf")
            nc.scalar.copy(f[:], nf[:])
            nc.sync.dma_start(out=taps["nf"][:], in_=f[:])

        # final AllGather of nf, then vocab-sharded logits
        nf_in = dram.tile([128, NFCOLS], BF16, tag="nfi")
        nf_out = dram.tile([G, 128, NFCOLS], BF16, tag="nfo")
        nc.sync.dma_start(out=nf_in[:], in_=nf[:])
        nc.gpsimd.collective_compute(
            "AllGather", ALU.bypass, replica_groups=REPLICA_GROUPS,
            ins=[nf_in.opt()], outs=[nf_out.opt()])
        nfg = hp.tile([128, G, KD, TPC], BF16, tag="h", name="nfg")
        for rho in range(G):
            nc.sync.dma_start(out=nfg[:, rho], in_=nf_out[rho])

        for vb in range(NVB):
            ebt = embp.tile([128, KD, 128], BF16, tag="emb", name=f"eb{vb}")
            nc.sync.dma_start(out=ebt[:], in_=inp["emb"][:, vb])
            for half in range(2):
                pl = ps_mm.tile([128, 512], F32, tag="mm",
                                name=f"pl{vb}_{half}")
                for k in range(KD):
                    nc.tensor.matmul(pl[:], ebt[:, k, :],
                                     nfg[:, ts(half, 2), k, :],
                                     start=(k == 0), stop=(k == KD - 1))
                lo = lout.tile([128, 512], BF16, tag="lo",
                               name=f"lo{vb}_{half}")
                if (vb + half) % 2 == 0:
                    nc.scalar.copy(lo[:], pl[:])
                else:
                    nc.vector.tensor_scalar_mul(lo[:], pl[:], 1.0)
                nc.sync.dma_start(out=logits[ts(vb, 128), ts(half, 512)],
                                  in_=lo[:])


# ------------------------------------------------------------------
# Host side
# ------------------------------------------------------------------

def _kfold(w):
    """[in, out] -> [128, in//128, out]."""
    i, o = w.shape
    return np.ascontiguousarray(w.reshape(i // 128, 128, o).transpose(1, 0, 2))


def _mslice(w):
    """[in, out] -> [128, out//128, in//128, 128] contiguous strips."""
    i, o = w.shape
    t = w.reshape(i // 128, 128, o // 128, 128)
    return np.ascontiguousarray(t.transpose(1, 2, 0, 3))


def _cols(v):
    """[n] -> [128, n//128] per-partition bias columns."""
    return np.ascontiguousarray(v.reshape(-1, 128).T)


def _bf(a):
    return np.ascontiguousarray(a).astype(BF16NP)


def prep_inputs(inputs):
    f = lambda a: np.asarray(a, np.float32)
    tokens = np.asarray(inputs["tokens"])
    tok_emb, pos_emb = f(inputs["tok_emb"]), f(inputs["pos_emb"])
    ln1_g = f(inputs["ln1_g"])
    wq, wk = f(inputs["wq"]), f(inputs["wk"])
    wv, wo = f(inputs["wv"]), f(inputs["wo"])
    ln2_g, ln2_b = f(inputs["ln2_g"]), f(inputs["ln2_b"])
    w1, b1 = f(inputs["w1"]), f(inputs["b1"])
    w2, b2 = f(inputs["w2"]), f(inputs["b2"])
    lnf_g = f(inputs["lnf_g"])

    sc = 1.0 / np.sqrt(HD)
    x0 = tok_emb[tokens] + pos_emb[:S][None]          # [B, S, D]
    ones = np.ones((128, 1), np.float32)

    # shared (identical on all cores) weight tensors
    shared = {"ones": ones}
    for l in range(L):
        shared[_f("wq", l)] = _bf(_mslice(ln1_g[l][:, None] * wq[l] * sc))
        shared[_f("wk", l)] = _bf(_mslice(ln1_g[l][:, None] * wk[l]))
        shared[_f("nks", l)] = _cols(-np.asarray(
            _bf(ln1_g[l][:, None] * wk[l]), np.float32).sum(0))
        shared[_f("wv", l)] = _bf(_kfold(ln1_g[l][:, None] * wv[l]))
        shared[_f("wo", l)] = _bf(_mslice(wo[l]))
        shared[_f("w1", l)] = _bf(_mslice(ln2_g[l][:, None] * w1[l]))
        shared[_f("w2", l)] = _bf(_mslice(w2[l]))
        shared[_f("b1", l)] = _cols(b1[l] + ln2_b[l] @ w1[l])
        shared[_f("b2", l)] = _cols(b2[l])

    in_maps = []
    for core in range(N_CORES):
        g, r = core // G, core % G
        A_blk, B_blk = r, 7 - r
        m = dict(shared)
        xo = np.concatenate([x0[g, 128 * A_blk:128 * A_blk + 128],
                             x0[g, 128 * B_blk:128 * B_blk + 128]], 0)
        m["x0"] = _kfold(np.ascontiguousarray(xo.T))
        m01 = np.zeros((128, NSLOT, BLK), np.float32)
        kp = np.arange(128)[:, None]
        qf = np.arange(128)[None, :]
        for s in range(NSLOT):
            qb = A_blk if s < 4 else B_blk
            kb = s if s < 4 else s - 4
            m01[:, s, :] = (128 * kb + kp <= 128 * qb + qf)
        m["m01"] = _bf(m01)
        v0 = r * VS
        v1 = min(v0 + VS, V)
        epad = np.zeros((D, VSP), np.float32)
        epad[:, :v1 - v0] = (tok_emb[v0:v1] * lnf_g[None, :]).T
        m["emb"] = _bf(_mslice(epad))
        in_maps.append(m)
    return in_maps


_CACHED = {}


def _get_program(debug_taps=False):
    key = bool(debug_taps)
    if key not in _CACHED:
        _CACHED[key] = build_program(debug_taps)
    return _CACHED[key]


def run(inputs, debug_taps=False, trace=False, **kw):
    nc = _get_program(debug_taps)
    in_maps = prep_inputs(inputs)
    return run_bass_kernel_spmd(nc, in_maps, list(range(N_CORES)),
                                trace=trace, **kw)


# token column -> natural token index within a group's 1024 tokens
def _colperm():
    perm = np.empty(T, np.int64)
    for c in range(T):
        rho, rem = divmod(c, 256)
        half, qf = divmod(rem, 128)
        blkid = rho if half == 0 else 7 - rho
        perm[c] = 128 * blkid + qf
    return perm


def assemble(results, inputs):
    lnf_b = np.asarray(inputs["lnf_b"], np.float32)
    tok_emb = np.asarray(inputs["tok_emb"], np.float32)
    perm = _colperm()
    out = np.empty((B, S, V), np.float32)
    for b in range(B):
        for r in range(G):
            v0 = r * VS
            v1 = min(v0 + VS, V)
            part = results[b * G + r]["logits"][:v1 - v0, :]  # [rows, T]
            out[b, perm, v0:v1] = part.T.astype(np.float32)
    if np.any(lnf_b):
        out += (tok_emb @ lnf_b)[None, None, :]
    return out


def kernel(**inputs):
    res = run(inputs)
    return assemble(res.results, inputs)


if __name__ == "__main__":
    print("building program...")
    build_program()
    print("build + compile OK")


# revision 35
# speedup vs baseline: 1.0653x; 1.0653x over previous
"""GPT forward pass on 8 Trainium2 NeuronCores — sequence-parallel (SP8).

Model: B=2, S=1024, D=1024, H=16 heads (hd=64), L=6 layers, V=50257,
tied embedding head.

Sharding: the 2048 tokens are split into 16 causal blocks of 128;
core c (group g=c//4 over batch, rank r=c%4) owns query blocks
A=r and B=7-r of batch g (256 tokens), which balances causal attention
work.  Every core holds the FULL weights (bf16); the only per-layer
communication is a bf16 AllGather of K then V (~0.5 MB each) within
each 4-core group.  The final LN output is AllGathered once before the
vocab-sharded tied-logit matmul.

Perf structure (v2):
- Deferred LayerNorm: projections contract x~ = x - mean directly
  (built with one vector op per strip); the 1/sigma scale is folded
  into the PSUM evictions.  This removes the serial LN-finish chain
  between FFN and the next layer's QKV projections, so the K/V
  AllGather triggers earlier.
- Attention holds all 16 heads' exp(scores) tiles so the serialized
  V AllGather hides behind phase-1 (scores+exp) work.
- Softmax normalization is batched: denominators are copied into one
  row, one reciprocal_approx_fast over [1, 4096], broadcast back via
  tiny ones-matmuls, and applied with 8 vector ops (instead of 32
  slow [1,128] reciprocals + gpsimd broadcasts).
- Logits are emitted in bf16 (halves the output DMA) and upcast on
  the host.
"""

import sys

sys.path.insert(0, "/opt/trn_rl_repo")

import contextlib

import numpy as np
import ml_dtypes

import concourse.bacc as bacc
import concourse.mybir as mybir
import concourse.tile as tile
from concourse.bass import ts
from concourse.bass_utils import run_bass_kernel_spmd

F32 = mybir.dt.float32
F32R = mybir.dt.float32r
BF16 = mybir.dt.bfloat16
AF = mybir.ActivationFunctionType
ALU = mybir.AluOpType
BF16NP = ml_dtypes.bfloat16

# Model dims
B, S, D, H, L, V = 2, 1024, 1024, 16, 6, 50257
HD = D // H            # 64
DFF = 4 * D            # 4096
N_CORES = 8
G = 4                  # group size (cores per batch element)
KD = D // 128          # 8 feature tiles
HC = H // 2            # 8 head-chunks (2 heads per 128 partitions)
TPC = 256              # tokens per core
BLK = 128              # token block
M1 = DFF // 128        # 32 w1 out strips
M2 = KD                # 8 w2 out strips
NSLOT = 12             # attention slots per head (4 A-side + 8 B-side)
VS = 12565             # vocab rows per group-rank (last: 12562)
VSP = 12800            # padded
NVB = VSP // 128       # 100 vocab blocks
T = 1024               # tokens per group (gathered)

KCOLS = HC * TPC            # 2048 k cols in kv contribution
VSEG = H * (HD + 1)         # 1040 v cols per token block (ones col incl.)
NFCOLS = KD * TPC           # 2048

# key block kb -> (rank, slot-within-rank)
RHO = [kb if kb < 4 else 7 - kb for kb in range(8)]
SIG = [0 if kb < 4 else 1 for kb in range(8)]

REPLICA_GROUPS = [[0, 1, 2, 3], [4, 5, 6, 7]]


def _f(name, l=None):
    return name if l is None else f"{name}{l}"


def build_program(debug_taps=False):
    nc = bacc.Bacc("TRN2", target_bir_lowering=False, debug=False,
                   enable_asserts=True, num_devices=N_CORES)

    inp = {}

    def dram_in(name, shape, dtype=BF16):
        inp[name] = nc.dram_tensor(name, shape, dtype, kind="ExternalInput").ap()
        return inp[name]

    dram_in("x0", [128, KD, TPC], F32R)
    dram_in("ones", [128, 1], F32R)
    dram_in("m01", [128, NSLOT, BLK], BF16)
    for l in range(L):
        dram_in(_f("wq", l), [128, KD, KD, 128])    # [p, m, kt, 128]
        dram_in(_f("wk", l), [128, KD, KD, 128])
        dram_in(_f("wv", l), [128, KD, D])          # plain k-fold (moving)
        dram_in(_f("wo", l), [128, KD, KD, 128])
        dram_in(_f("w1", l), [128, M1, KD, 128])
        dram_in(_f("w2", l), [128, M2, M1, 128])
        dram_in(_f("b1", l), [128, M1], F32)
        dram_in(_f("b2", l), [128, M2], F32)
        dram_in(_f("nks", l), [128, KD], F32)       # -colsum(wk), per strip
    dram_in("emb", [128, NVB, KD, 128])
    logits = nc.dram_tensor("logits", [VSP, T], BF16, kind="ExternalOutput").ap()

    taps = {}
    if debug_taps:
        for name in ["xa_0", "x_1", "nf"]:
            taps[name] = nc.dram_tensor("dbg_" + name, [128, KD, TPC], F32,
                                        kind="ExternalOutput").ap()

    with tile.TileContext(nc) as tc:
        _body(tc, inp, logits, taps)
    nc.compile()
    return nc


def _body(tc, inp, logits, taps):
    nc = tc.nc
    ctx = contextlib.ExitStack()
    with ctx:
        # --- SBUF pools ---
        singles = ctx.enter_context(tc.tile_pool(name="singles", bufs=1))
        xp = ctx.enter_context(tc.tile_pool(name="xp", bufs=1))        # 8K
        npool = ctx.enter_context(tc.tile_pool(name="npool", bufs=2))  # 8K
        tmp = ctx.enter_context(tc.tile_pool(name="tmp", bufs=3))      # 3K
        qp = ctx.enter_context(tc.tile_pool(name="qp", bufs=1))        # 4K
        stg = ctx.enter_context(tc.tile_pool(name="stg", bufs=1))      # 8.25K
        kvp = ctx.enter_context(tc.tile_pool(name="kvp", bufs=1))      # 33K
        op = ctx.enter_context(tc.tile_pool(name="op", bufs=1))        # 4K
        hp = ctx.enter_context(tc.tile_pool(name="hp", bufs=1))        # 16K
        wq_p = ctx.enter_context(tc.tile_pool(name="wq_p", bufs=8))    # 16K
        wv_p = ctx.enter_context(tc.tile_pool(name="wv_p", bufs=4))    # 4K
        w2_p = ctx.enter_context(tc.tile_pool(name="w2_p", bufs=3))    # 24K
        embp = ctx.enter_context(tc.tile_pool(name="embp", bufs=2))    # 4K
        ep = ctx.enter_context(tc.tile_pool(name="ep", bufs=16))       # 48K
        stat = ctx.enter_context(tc.tile_pool(name="stat", bufs=2))
        bcp = ctx.enter_context(tc.tile_pool(name="bcp", bufs=2))      # 2K
        lout = ctx.enter_context(tc.tile_pool(name="lout", bufs=2))    # 2K
        bias = ctx.enter_context(tc.tile_pool(name="bias", bufs=2))
        # --- PSUM pools (8 banks) ---
        ps_mm = ctx.enter_context(tc.tile_pool(name="ps_mm", bufs=4, space="PSUM"))
        ps_po = ctx.enter_context(tc.tile_pool(name="ps_po", bufs=2, space="PSUM"))
        ps_st = ctx.enter_context(tc.tile_pool(name="ps_st", bufs=1, space="PSUM"))
        # --- DRAM (collective bounce) ---
        dram = ctx.enter_context(tc.tile_pool(name="dram", bufs=2, space="DRAM"))

        # --- constants / persistent ---
        ones_t = singles.tile([128, 1], F32R)
        nc.sync.dma_start(out=ones_t[:], in_=inp["ones"][:])
        eps_t = singles.tile([1, 1], F32)
        nc.vector.memset(eps_t[:], 1e-5)
        m01t = singles.tile([128, NSLOT, BLK], BF16)
        nc.sync.dma_start(out=m01t[:], in_=inp["m01"][:])

        xt = xp.tile([128, KD, TPC], F32R, tag="x")
        nc.sync.dma_start(out=xt[:], in_=inp["x0"][:])

        def ln_alloc(nm):
            s1 = ps_st.tile([1, TPC], F32, tag="st1", name=f"s1{nm}")
            s2 = ps_st.tile([1, TPC], F32, tag="st2", name=f"s2{nm}")
            return s1, s2

        def ln_stats_k(src, s1, s2, k):
            nc.tensor.matmul(s1[:], ones_t[:], src[:, k, :],
                             start=(k == 0), stop=(k == KD - 1))
            sq = tmp.tile([128, TPC], F32R, tag="tmp", name=f"sq_{k}")
            nc.vector.tensor_tensor(
                out=sq[:], in0=src[:, k, :].bitcast(F32),
                in1=src[:, k, :].bitcast(F32), op=ALU.mult)
            nc.tensor.matmul(s2[:], ones_t[:], sq[:],
                             start=(k == 0), stop=(k == KD - 1))

        def mean_chain(s1, nm):
            """s1 -> (m [1,TPC] f32, mB_s [128,TPC] f32 in SBUF)."""
            m = stat.tile([1, TPC], F32, tag="sa", name=f"m{nm}")
            nc.vector.tensor_scalar_mul(m[:], s1[:], 1.0 / D)
            mB_s = bcp.tile([128, TPC], F32, tag="mb", name=f"mb{nm}")
            nc.gpsimd.partition_broadcast(mB_s[:], m[:])
            return m, mB_s

        def xtilde(mB_s, nm):
            """x~ = x - mean, cast to bf16 (one op per strip)."""
            xtl = npool.tile([128, KD, TPC], BF16, tag="n", name=f"xt{nm}")
            for k in range(KD):
                nc.vector.tensor_tensor(
                    out=xtl[:, k, :], in0=xt[:, k, :].bitcast(F32),
                    in1=mB_s[:], op=ALU.subtract)
            return xtl

        def rs_chain(s2, m, nm, want_col=False):
            """-> rs_bs [128,TPC] f32 in SBUF (and rs_col [128,2] if asked)."""
            msq = stat.tile([1, TPC], F32, tag="sb", name=f"msq{nm}")
            nc.vector.tensor_tensor(out=msq[:], in0=m[:].bitcast(F32),
                                    in1=m[:].bitcast(F32), op=ALU.mult)
            var = stat.tile([1, TPC], F32, tag="sb", name=f"var{nm}")
            nc.vector.scalar_tensor_tensor(
                out=var[:], in0=s2[:], scalar=1.0 / D, in1=msq[:],
                op0=ALU.mult, op1=ALU.subtract)
            var2 = stat.tile([1, TPC], F32R, tag="sb", name=f"var2{nm}")
            nc.vector.tensor_scalar_add(var2[:], var[:], 1e-5)
            vr = stat.tile([1, TPC], F32, tag="sb", name=f"vr{nm}")
            nc.vector.reciprocal_approx_fast(vr[:], var2[:].bitcast(F32))
            rs = stat.tile([1, TPC], F32, tag="sb", name=f"rs{nm}")
            nc.scalar.activation(rs[:], vr[:], AF.Sqrt)
            rs_bs = bcp.tile([128, TPC], F32, tag="rs", name=f"rsb{nm}")
            nc.gpsimd.partition_broadcast(rs_bs[:], rs[:])
            return rs_bs

        def k_stage_gather(l, xb, mB_s, rs_bs, nks_t):
            """K = (wk^T xb - ksum*m) * rs -> staging -> AllGather.

            The matmuls contract the raw (uncentered) bf16 residual xb,
            so they queue immediately behind the previous FFN with no
            LN-stats dependency; the mean/scale fixup rides the PSUM
            eviction.  Staging DMAs go out per strip."""
            stage = stg.tile([128, KCOLS], BF16, tag="stgk", name=f"stagek{l}")
            ag_in = dram.tile([128, KCOLS], BF16, tag="agik", name=f"agik{l}")
            ag_out = dram.tile([G, 128, KCOLS], BF16, tag="agok",
                               name=f"agok{l}")
            for mp in range(KD // 2):
                pk = ps_mm.tile([128, 2, TPC], F32, tag="mm",
                                name=f"pk{l}_{mp}")
                for j in range(2):
                    m = 2 * mp + j
                    wkt = wq_p.tile([128, KD, 128], BF16, tag="wqk",
                                    name=f"wk{l}_{m}")
                    nc.sync.dma_start(out=wkt[:], in_=inp[_f("wk", l)][:, m])
                    for k in range(KD):
                        nc.tensor.matmul(pk[:, j, :], wkt[:, k, :],
                                         xb[:, k, :],
                                         start=(k == 0), stop=(k == KD - 1))
                    t = tmp.tile([128, TPC], F32, tag="tmp",
                                 name=f"kt{l}_{m}")
                    nc.vector.scalar_tensor_tensor(
                        out=t[:], in0=mB_s[:], scalar=nks_t[:, m:m + 1],
                        in1=pk[:, j, :], op0=ALU.mult, op1=ALU.add)
                    nc.vector.tensor_tensor(
                        out=stage[:, ts(m, TPC)], in0=t[:],
                        in1=rs_bs[:], op=ALU.mult)
                    nc.sync.dma_start(out=ag_in[:, ts(m, TPC)],
                                      in_=stage[:, ts(m, TPC)])
            nc.gpsimd.collective_compute(
                "AllGather", ALU.bypass, replica_groups=REPLICA_GROUPS,
                ins=[ag_in.opt()], outs=[ag_out.opt()])
            return ag_out

        def v_stage_gather(l, n1v):
            """V (token-major, + ones cols) -> staging -> AllGather."""
            stage = stg.tile([128, 2 * VSEG], BF16, tag="stgv",
                             name=f"stagev{l}")
            ones_view = stage[:].rearrange(
                "p (b h c) -> p b h c", b=2, h=H)[:, :, :, HD:]
            nc.vector.memset(ones_view, 1.0)
            for hh in range(2):   # halves of the head dim (512 cols)
                pva = ps_po.tile([128, 512], F32, tag="po",
                                 name=f"pva{l}_{hh}")
                pvb = ps_po.tile([128, 512], F32, tag="po",
                                 name=f"pvb{l}_{hh}")
                for k in range(KD):
                    wvs = wv_p.tile([128, 512], BF16, tag="wv",
                                    name=f"wv{l}_{hh}_{k}")
                    nc.sync.dma_start(out=wvs[:],
                                      in_=inp[_f("wv", l)][:, k, ts(hh, 512)])
                    nc.tensor.matmul(pva[:], n1v[:, k, ts(0, BLK)], wvs[:],
                                     start=(k == 0), stop=(k == KD - 1))
                    nc.tensor.matmul(pvb[:], n1v[:, k, ts(1, BLK)], wvs[:],
                                     start=(k == 0), stop=(k == KD - 1))
                for blk, pv in ((0, pva), (1, pvb)):
                    for j in range(8):
                        h = hh * 8 + j
                        nc.vector.tensor_scalar_mul(
                            stage[:, blk * VSEG + h * (HD + 1):
                                  blk * VSEG + h * (HD + 1) + HD],
                            pv[:, ts(j, HD)], 1.0)
            ag_in = dram.tile([128, 2 * VSEG], BF16, tag="agiv",
                              name=f"agiv{l}")
            ag_out = dram.tile([G, 128, 2 * VSEG], BF16, tag="agov",
                               name=f"agov{l}")
            nc.sync.dma_start(out=ag_in[:], in_=stage[:])
            nc.gpsimd.collective_compute(
                "AllGather", ALU.bypass, replica_groups=REPLICA_GROUPS,
                ins=[ag_in.opt()], outs=[ag_out.opt()])
            return ag_out

        def q_proj(l, xtl, rs_bs):
            q = qp.tile([128, KD, TPC], BF16, tag="q", name=f"q{l}")
            for mp in range(KD // 2):
                pq = ps_mm.tile([128, 2, TPC], F32, tag="mm",
                                name=f"pq{l}_{mp}")
                for j in range(2):
                    m = 2 * mp + j
                    wqt = wq_p.tile([128, KD, 128], BF16, tag="wqk",
                                    name=f"wq{l}_{m}")
                    nc.sync.dma_start(out=wqt[:], in_=inp[_f("wq", l)][:, m])
                    for k in range(KD):
                        nc.tensor.matmul(pq[:, j, :], wqt[:, k, :],
                                         xtl[:, k, :],
                                         start=(k == 0), stop=(k == KD - 1))
                    nc.vector.tensor_tensor(
                        out=q[:, m, :], in0=pq[:, j, :], in1=rs_bs[:],
                        op=ALU.mult)
            return q

        def attention(l, q, kg, vg, oT):

            def phase1(h):
                pp = 64 * (h % 2)
                hc = h // 2
                eTile = ep.tile([128, NSLOT, BLK], BF16, tag="eT",
                                name=f"eT{l}_{h}")
                # scores (transposed [k, q]) + exp + causal mask,
                # batched 4 slots per PSUM bank
                for grp in range(NSLOT // 4):
                    pss = ps_mm.tile([128, 4, BLK], F32, tag="mm",
                                     name=f"sc{l}_{h}_{grp}")
                    for j in range(4):
                        s = 4 * grp + j
                        kb = s if s < 4 else s - 4
                        qc = ts(0, BLK) if s < 4 else ts(1, BLK)
                        rho, sg = RHO[kb], SIG[kb]
                        nc.tensor.matmul(
                            pss[:, j, :],
                            kg[pp:pp + 64, rho,
                               hc * TPC + sg * BLK: hc * TPC + sg * BLK + BLK],
                            q[pp:pp + 64, hc, qc], start=True, stop=True)
                    nc.scalar.activation(eTile[:, ts(grp, 4), :], pss[:],
                                         AF.Exp)
                    nc.vector.tensor_tensor(
                        out=eTile[:, ts(grp, 4), :],
                        in0=eTile[:, ts(grp, 4), :],
                        in1=m01t[:, ts(grp, 4), :], op=ALU.mult)
                return eTile

            def phase2(h, eTile):
                pp = 64 * (h % 2)
                hc = h // 2
                # AV (+ ones-column denominator); A-side (cols 0:128) and
                # B-side (cols 128:256) are two accumulation groups in ONE
                # bank — the B start=True clears only has_written bits, the
                # already-final A values persist.
                pso = ps_po.tile([HD + 1, TPC], F32, tag="po",
                                 name=f"pav{l}_{h}")
                for s in range(NSLOT):
                    kb = s if s < 4 else s - 4
                    rho, sg = RHO[kb], SIG[kb]
                    cc = ts(0, BLK) if s < 4 else ts(1, BLK)
                    vsl = vg[:, rho,
                             sg * VSEG + h * (HD + 1):
                             sg * VSEG + (h + 1) * (HD + 1)]
                    nc.tensor.matmul(pso[:, cc], vsl, eTile[:, s, :],
                                     start=(s in (0, 4)),
                                     stop=(s in (3, NSLOT - 1)),
                                     skip_group_check=True)
                # free the PSUM slot fast: copy den row + raw numerators,
                # then normalize off-PSUM with the 1-pass approx reciprocal
                dh = stat.tile([1, TPC], F32, tag="rr", name=f"dh{l}_{h}")
                nc.vector.tensor_scalar_mul(dh[:], pso[HD:HD + 1, :], 1.0)
                nc.vector.tensor_scalar_mul(
                    oT[pp:pp + 64, hc, :], pso[0:HD, :], 1.0)
                rh = stat.tile([1, TPC], F32, tag="rr", name=f"rh{l}_{h}")
                nc.vector.reciprocal_approx_fast(rh[:], dh[:])
                bb = bcp.tile([128, TPC], F32, tag="bb", name=f"bb{l}_{h}")
                nc.gpsimd.partition_broadcast(bb[:], rh[:])
                nc.vector.tensor_tensor(
                    out=oT[pp:pp + 64, hc, :], in0=oT[pp:pp + 64, hc, :],
                    in1=bb[pp:pp + 64, :], op=ALU.mult)

            held = [phase1(h) for h in range(H)]
            for h in range(H):
                phase2(h, held[h])

        def wo_residual(l, oT, s1, s2):
            for mp in range(KD // 2):
                po = ps_mm.tile([128, 2, TPC], F32, tag="mm",
                                name=f"po{l}_{mp}")
                for j in range(2):
                    m = 2 * mp + j
                    wot = wq_p.tile([128, KD, 128], BF16, tag="wqk",
                                    name=f"wo{l}_{m}")
                    nc.sync.dma_start(out=wot[:], in_=inp[_f("wo", l)][:, m])
                    for k in range(KD):
                        nc.tensor.matmul(po[:, j, :], wot[:, k, :],
                                         oT[:, k, :],
                                         start=(k == 0), stop=(k == KD - 1))
                    nc.vector.tensor_tensor(
                        out=xt[:, m, :], in0=xt[:, m, :].bitcast(F32),
                        in1=po[:, j, :], op=ALU.add)
                for j in range(2):
                    ln_stats_k(xt, s1, s2, 2 * mp + j)

        def ffn(l, xtl2, rs_bs2, s1, s2, xb_next):
            b1_t = bias.tile([128, M1], F32, tag="bias", name=f"b1{l}")
            nc.sync.dma_start(out=b1_t[:], in_=inp[_f("b1", l)][:])
            b2_t = bias.tile([128, M2], F32, tag="bias", name=f"b2{l}")
            nc.sync.dma_start(out=b2_t[:], in_=inp[_f("b2", l)][:])
            hT = hp.tile([128, M1, TPC], BF16, tag="h", name=f"hT{l}")
            for mp in range(M1 // 2):
                p1 = ps_mm.tile([128, 2, TPC], F32, tag="mm",
                                name=f"p1{l}_{mp}")
                for j in range(2):
                    m = 2 * mp + j
                    w1t = wq_p.tile([128, KD, 128], BF16, tag="wqk",
                                    name=f"w1{l}_{m}")
                    nc.sync.dma_start(out=w1t[:], in_=inp[_f("w1", l)][:, m])
                    for k in range(KD):
                        nc.tensor.matmul(p1[:, j, :], w1t[:, k, :],
                                         xtl2[:, k, :],
                                         start=(k == 0), stop=(k == KD - 1))
                    u = tmp.tile([128, TPC], F32, tag="tmp",
                                 name=f"u{l}_{m}")
                    nc.vector.tensor_tensor(
                        out=u[:], in0=p1[:, j, :], in1=rs_bs2[:],
                        op=ALU.mult)
                    nc.scalar.activation(hT[:, m, :], u[:], AF.Gelu,
                                         bias=b1_t[:, m:m + 1])
            for mp in range(M2 // 2):
                p2 = ps_mm.tile([128, 2, TPC], F32, tag="mm",
                                name=f"p2{l}_{mp}")
                for j in range(2):
                    m = 2 * mp + j
                    w2t = w2_p.tile([128, M1, 128], BF16, tag="w2",
                                    name=f"w2{l}_{m}")
                    nc.sync.dma_start(out=w2t[:], in_=inp[_f("w2", l)][:, m])
                    for k in range(M1):
                        nc.tensor.matmul(p2[:, j, :], w2t[:, k, :],
                                         hT[:, k, :],
                                         start=(k == 0), stop=(k == M1 - 1))
                    nc.vector.scalar_tensor_tensor(
                        out=xt[:, m, :], in0=p2[:, j, :],
                        scalar=b2_t[:, m:m + 1],
                        in1=xt[:, m, :].bitcast(F32), op0=ALU.add,
                        op1=ALU.add)
                    nc.scalar.copy(xb_next[:, m, :],
                                   xt[:, m, :].bitcast(F32))
                for j in range(2):
                    ln_stats_k(xt, s1, s2, 2 * mp + j)

        def tap_f32(name):
            if name in taps:
                nc.sync.dma_start(out=taps[name][:], in_=xt[:].bitcast(F32))

        # ---------------- main loop ----------------
        s1, s2 = ln_alloc("ln_0")
        xb = npool.tile([128, KD, TPC], BF16, tag="n", name="xb0")
        for k in range(KD):
            ln_stats_k(xt, s1, s2, k)
            nc.scalar.copy(xb[:, k, :], xt[:, k, :].bitcast(F32))
        for l in range(L):
            nks_t = bias.tile([128, KD], F32, tag="nks", name=f"nks{l}")
            nc.sync.dma_start(out=nks_t[:], in_=inp[_f("nks", l)][:])
            m, mB_s = mean_chain(s1, f"l{l}")
            rs_bs = rs_chain(s2, m, f"l{l}")
            agk = k_stage_gather(l, xb, mB_s, rs_bs, nks_t)
            xtl = xtilde(mB_s, f"l{l}")
            n1v = npool.tile([128, KD, TPC], BF16, tag="n", name=f"n1v{l}")
            for k in range(KD):
                nc.vector.tensor_tensor(
                    out=n1v[:, k, :], in0=xtl[:, k, :], in1=rs_bs[:],
                    op=ALU.mult)
            agv = v_stage_gather(l, n1v)
            q = q_proj(l, xtl, rs_bs)
            kg = kvp.tile([128, G, KCOLS], BF16, tag="kg", name=f"kg{l}")
            for rho in range(G):
                nc.sync.dma_start(out=kg[:, rho, :], in_=agk[rho])
            vg = kvp.tile([128, G, 2 * VSEG], BF16, tag="vg", name=f"vg{l}")
            for rho in range(G):
                nc.sync.dma_start(out=vg[:, rho, :], in_=agv[rho])
            oT = op.tile([128, KD, TPC], BF16, tag="oT", name=f"oT{l}")
            attention(l, q, kg, vg, oT)
            s1a, s2a = ln_alloc(f"ln2_{l}")
            wo_residual(l, oT, s1a, s2a)
            if l == 0:
                tap_f32("xa_0")
            m2, m2B_s = mean_chain(s1a, f"f{l}")
            xtl2 = xtilde(m2B_s, f"f{l}")
            rs_bs2 = rs_chain(s2a, m2, f"f{l}")
            s1, s2 = ln_alloc(f"ln1_{l + 1}")
            xb = npool.tile([128, KD, TPC], BF16, tag="n", name=f"xb{l + 1}")
            ffn(l, xtl2, rs_bs2, s1, s2, xb)
            if l == 0:
                tap_f32("x_1")

        # final LN -> nf
        mf, mfB_s = mean_chain(s1, "fin")
        xtlf = xtilde(mfB_s, "fin")
        rs_bsf = rs_chain(s2, mf, "fin")
        nf = npool.tile([128, KD, TPC], BF16, tag="n", name="nf")
        for k in range(KD):
            nc.vector.tensor_tensor(
                out=nf[:, k, :], in0=xtlf[:, k, :], in1=rs_bsf[:],
                op=ALU.mult)
        if "nf" in taps:
            f = stg.tile([128, KD, TPC], F32, tag="tapf", name="tpnf")
            nc.scalar.copy(f[:], nf[:])
            nc.sync.dma_start(out=taps["nf"][:], in_=f[:])

        # final AllGather of nf, then vocab-sharded logits
        nf_in = dram.tile([128, NFCOLS], BF16, tag="nfi")
        nf_out = dram.tile([G, 128, NFCOLS], BF16, tag="nfo")
        nc.sync.dma_start(out=nf_in[:], in_=nf[:])
        nc.gpsimd.collective_compute(
            "AllGather", ALU.bypass, replica_groups=REPLICA_GROUPS,
            ins=[nf_in.opt()], outs=[nf_out.opt()])
        nfg = hp.tile([128, G, KD, TPC], BF16, tag="h", name="nfg")
        for rho in range(G):
            nc.sync.dma_start(out=nfg[:, rho], in_=nf_out[rho])

        for vb in range(NVB):
            ebt = embp.tile([128, KD, 128], BF16, tag="emb", name=f"eb{vb}")
            nc.sync.dma_start(out=ebt[:], in_=inp["emb"][:, vb])
            for half in range(2):
                pl = ps_mm.tile([128, 512], F32, tag="mm",
                                name=f"pl{vb}_{half}")
                for k in range(KD):
                    nc.tensor.matmul(pl[:], ebt[:, k, :],
                                     nfg[:, ts(half, 2), k, :],
                                     start=(k == 0), stop=(k == KD - 1))
                lo = lout.tile([128, 512], BF16, tag="lo",
                               name=f"lo{vb}_{half}")
                if (vb + half) % 2 == 0:
                    nc.scalar.copy(lo[:], pl[:])
                else:
                    nc.vector.tensor_scalar_mul(lo[:], pl[:], 1.0)
                nc.sync.dma_start(out=logits[ts(vb, 128), ts(half, 512)],
                                  in_=lo[:])


# ------------------------------------------------------------------
# Host side
# ------------------------------------------------------------------

def _kfold(w):
    """[in, out] -> [128, in//128, out]."""
    i, o = w.shape
    return np.ascontiguousarray(w.reshape(i // 128, 128, o).transpose(1, 0, 2))


def _mslice(w):
    """[in, out] -> [128, out//128, in//128, 128] contiguous strips."""
    i, o = w.shape
    t = w.reshape(i // 128, 128, o // 128, 128)
    return np.ascontiguousarray(t.transpose(1, 2, 0, 3))


def _cols(v):
    """[n] -> [128, n//128] per-partition bias columns."""
    return np.ascontiguousarray(v.reshape(-1, 128).T)


def _bf(a):
    return np.ascontiguousarray(a).astype(BF16NP)


def prep_inputs(inputs):
    f = lambda a: np.asarray(a, np.float32)
    tokens = np.asarray(inputs["tokens"])
    tok_emb, pos_emb = f(inputs["tok_emb"]), f(inputs["pos_emb"])
    ln1_g = f(inputs["ln1_g"])
    wq, wk = f(inputs["wq"]), f(inputs["wk"])
    wv, wo = f(inputs["wv"]), f(inputs["wo"])
    ln2_g, ln2_b = f(inputs["ln2_g"]), f(inputs["ln2_b"])
    w1, b1 = f(inputs["w1"]), f(inputs["b1"])
    w2, b2 = f(inputs["w2"]), f(inputs["b2"])
    lnf_g = f(inputs["lnf_g"])

    sc = 1.0 / np.sqrt(HD)
    x0 = tok_emb[tokens] + pos_emb[:S][None]          # [B, S, D]
    ones = np.ones((128, 1), np.float32)

    # shared (identical on all cores) weight tensors
    shared = {"ones": ones}
    for l in range(L):
        shared[_f("wq", l)] = _bf(_mslice(ln1_g[l][:, None] * wq[l] * sc))
        shared[_f("wk", l)] = _bf(_mslice(ln1_g[l][:, None] * wk[l]))
        shared[_f("nks", l)] = _cols(-np.asarray(
            _bf(ln1_g[l][:, None] * wk[l]), np.float32).sum(0))
        shared[_f("wv", l)] = _bf(_kfold(ln1_g[l][:, None] * wv[l]))
        shared[_f("wo", l)] = _bf(_mslice(wo[l]))
        shared[_f("w1", l)] = _bf(_mslice(ln2_g[l][:, None] * w1[l]))
        shared[_f("w2", l)] = _bf(_mslice(w2[l]))
        shared[_f("b1", l)] = _cols(b1[l] + ln2_b[l] @ w1[l])
        shared[_f("b2", l)] = _cols(b2[l])

    in_maps = []
    for core in range(N_CORES):
        g, r = core // G, core % G
        A_blk, B_blk = r, 7 - r
        m = dict(shared)
        xo = np.concatenate([x0[g, 128 * A_blk:128 * A_blk + 128],
                             x0[g, 128 * B_blk:128 * B_blk + 128]], 0)
        m["x0"] = _kfold(np.ascontiguousarray(xo.T))
        m01 = np.zeros((128, NSLOT, BLK), np.float32)
        kp = np.arange(128)[:, None]
        qf = np.arange(128)[None, :]
        for s in range(NSLOT):
            qb = A_blk if s < 4 else B_blk
            kb = s if s < 4 else s - 4
            m01[:, s, :] = (128 * kb + kp <= 128 * qb + qf)
        m["m01"] = _bf(m01)
        v0 = r * VS
        v1 = min(v0 + VS, V)
        epad = np.zeros((D, VSP), np.float32)
        epad[:, :v1 - v0] = (tok_emb[v0:v1] * lnf_g[None, :]).T
        m["emb"] = _bf(_mslice(epad))
        in_maps.append(m)
    return in_maps


_CACHED = {}


def _get_program(debug_taps=False):
    key = bool(debug_taps)
    if key not in _CACHED:
        _CACHED[key] = build_program(debug_taps)
    return _CACHED[key]


def run(inputs, debug_taps=False, trace=False, **kw):
    nc = _get_program(debug_taps)
    in_maps = prep_inputs(inputs)
    return run_bass_kernel_spmd(nc, in_maps, list(range(N_CORES)),
                                trace=trace, **kw)


# token column -> natural token index within a group's 1024 tokens
def _colperm():
    perm = np.empty(T, np.int64)
    for c in range(T):
        rho, rem = divmod(c, 256)
        half, qf = divmod(rem, 128)
        blkid = rho if half == 0 else 7 - rho
        perm[c] = 128 * blkid + qf
    return perm


def assemble(results, inputs):
    lnf_b = np.asarray(inputs["lnf_b"], np.float32)
    tok_emb = np.asarray(inputs["tok_emb"], np.float32)
    perm = _colperm()
    out = np.empty((B, S, V), np.float32)
    for b in range(B):
        for r in range(G):
            v0 = r * VS
            v1 = min(v0 + VS, V)
            part = results[b * G + r]["logits"][:v1 - v0, :]  # [rows, T]
            out[b, perm, v0:v1] = part.T.astype(np.float32)
    if np.any(lnf_b):
        out += (tok_emb @ lnf_b)[None, None, :]
    return out


def kernel(**inputs):
    res = run(inputs)
    return assemble(res.results, inputs)


if __name__ == "__main__":
    print("building program...")
    build_program()
    print("build + compile OK")


# revision 36
# speedup vs baseline: 1.0839x; 1.0174x over previous
"""GPT forward pass on 8 Trainium2 NeuronCores — sequence-parallel (SP8).

Model: B=2, S=1024, D=1024, H=16 heads (hd=64), L=6 layers, V=50257,
tied embedding head.

Sharding: the 2048 tokens are split into 16 causal blocks of 128;
core c (group g=c//4 over batch, rank r=c%4) owns query blocks
A=r and B=7-r of batch g (256 tokens), which balances causal attention
work.  Every core holds the FULL weights (bf16); the only per-layer
communication is a bf16 AllGather of K then V (~0.5 MB each) within
each 4-core group.  The final LN output is AllGathered once before the
vocab-sharded tied-logit matmul.

Perf structure (v2):
- Deferred LayerNorm: projections contract x~ = x - mean directly
  (built with one vector op per strip); the 1/sigma scale is folded
  into the PSUM evictions.  This removes the serial LN-finish chain
  between FFN and the next layer's QKV projections, so the K/V
  AllGather triggers earlier.
- Attention holds all 16 heads' exp(scores) tiles so the serialized
  V AllGather hides behind phase-1 (scores+exp) work.
- Softmax normalization is batched: denominators are copied into one
  row, one reciprocal_approx_fast over [1, 4096], broadcast back via
  tiny ones-matmuls, and applied with 8 vector ops (instead of 32
  slow [1,128] reciprocals + gpsimd broadcasts).
- Logits are emitted in bf16 (halves the output DMA) and upcast on
  the host.
"""

import sys

sys.path.insert(0, "/opt/trn_rl_repo")

import contextlib

import numpy as np
import ml_dtypes

import concourse.bacc as bacc
import concourse.mybir as mybir
import concourse.tile as tile
from concourse.bass import ts
from concourse.bass_utils import run_bass_kernel_spmd

F32 = mybir.dt.float32
F32R = mybir.dt.float32r
BF16 = mybir.dt.bfloat16
AF = mybir.ActivationFunctionType
ALU = mybir.AluOpType
BF16NP = ml_dtypes.bfloat16

# Model dims
B, S, D, H, L, V = 2, 1024, 1024, 16, 6, 50257
HD = D // H            # 64
DFF = 4 * D            # 4096
N_CORES = 8
G = 4                  # group size (cores per batch element)
KD = D // 128          # 8 feature tiles
HC = H // 2            # 8 head-chunks (2 heads per 128 partitions)
TPC = 256              # tokens per core
BLK = 128              # token block
M1 = DFF // 128        # 32 w1 out strips
M2 = KD                # 8 w2 out strips
NSLOT = 12             # attention slots per head (4 A-side + 8 B-side)
VS = 12565             # vocab rows per group-rank (last: 12562)
VSP = 12800            # padded
NVB = VSP // 128       # 100 vocab blocks
T = 1024               # tokens per group (gathered)

KCOLS = HC * TPC            # 2048 k cols in kv contribution
VSEG = H * (HD + 1)         # 1040 v cols per token block (ones col incl.)
NFCOLS = KD * TPC           # 2048

# key block kb -> (rank, slot-within-rank)
RHO = [kb if kb < 4 else 7 - kb for kb in range(8)]
SIG = [0 if kb < 4 else 1 for kb in range(8)]

REPLICA_GROUPS = [[0, 1, 2, 3], [4, 5, 6, 7]]


def _f(name, l=None):
    return name if l is None else f"{name}{l}"


def build_program(debug_taps=False):
    nc = bacc.Bacc("TRN2", target_bir_lowering=False, debug=False,
                   enable_asserts=True, num_devices=N_CORES)

    inp = {}

    def dram_in(name, shape, dtype=BF16):
        inp[name] = nc.dram_tensor(name, shape, dtype, kind="ExternalInput").ap()
        return inp[name]

    dram_in("x0", [128, KD, TPC], F32R)
    dram_in("ones", [128, 1], F32R)
    dram_in("m01", [128, NSLOT, BLK], BF16)
    for l in range(L):
        dram_in(_f("wq", l), [128, KD, KD, 128])    # [p, m, kt, 128]
        dram_in(_f("wk", l), [128, KD, KD, 128])
        dram_in(_f("wv", l), [128, KD, D])          # plain k-fold (moving)
        dram_in(_f("wo", l), [128, KD, KD, 128])
        dram_in(_f("w1", l), [128, M1, KD, 128])
        dram_in(_f("w2", l), [128, M2, M1, 128])
        dram_in(_f("b1", l), [128, M1], F32)
        dram_in(_f("b2", l), [128, M2], F32)
        dram_in(_f("nks", l), [128, KD], F32)       # -colsum(wk), per strip
    dram_in("emb", [128, NVB, KD, 128])
    logits = nc.dram_tensor("logits", [VSP, T], BF16, kind="ExternalOutput").ap()

    taps = {}
    if debug_taps:
        for name in ["xa_0", "x_1", "nf"]:
            taps[name] = nc.dram_tensor("dbg_" + name, [128, KD, TPC], F32,
                                        kind="ExternalOutput").ap()

    with tile.TileContext(nc) as tc:
        _body(tc, inp, logits, taps)
    nc.compile()
    return nc


def _body(tc, inp, logits, taps):
    nc = tc.nc
    ctx = contextlib.ExitStack()
    with ctx:
        # --- SBUF pools ---
        singles = ctx.enter_context(tc.tile_pool(name="singles", bufs=1))
        xp = ctx.enter_context(tc.tile_pool(name="xp", bufs=1))        # 8K
        npool = ctx.enter_context(tc.tile_pool(name="npool", bufs=2))  # 8K
        tmp = ctx.enter_context(tc.tile_pool(name="tmp", bufs=3))      # 3K
        qp = ctx.enter_context(tc.tile_pool(name="qp", bufs=1))        # 4K
        stg = ctx.enter_context(tc.tile_pool(name="stg", bufs=1))      # 8.25K
        kvp = ctx.enter_context(tc.tile_pool(name="kvp", bufs=1))      # 33K
        op = ctx.enter_context(tc.tile_pool(name="op", bufs=1))        # 4K
        hp = ctx.enter_context(tc.tile_pool(name="hp", bufs=1))        # 16K
        wq_p = ctx.enter_context(tc.tile_pool(name="wq_p", bufs=8))    # 16K
        wv_p = ctx.enter_context(tc.tile_pool(name="wv_p", bufs=4))    # 4K
        w2_p = ctx.enter_context(tc.tile_pool(name="w2_p", bufs=3))    # 24K
        embp = ctx.enter_context(tc.tile_pool(name="embp", bufs=2))    # 4K
        ep = ctx.enter_context(tc.tile_pool(name="ep", bufs=16))       # 48K
        stat = ctx.enter_context(tc.tile_pool(name="stat", bufs=2))
        bcp = ctx.enter_context(tc.tile_pool(name="bcp", bufs=2))      # 2K
        lout = ctx.enter_context(tc.tile_pool(name="lout", bufs=2))    # 2K
        bias = ctx.enter_context(tc.tile_pool(name="bias", bufs=2))
        # --- PSUM pools (8 banks) ---
        ps_mm = ctx.enter_context(tc.tile_pool(name="ps_mm", bufs=4, space="PSUM"))
        ps_po = ctx.enter_context(tc.tile_pool(name="ps_po", bufs=2, space="PSUM"))
        ps_st = ctx.enter_context(tc.tile_pool(name="ps_st", bufs=1, space="PSUM"))
        # --- DRAM (collective bounce) ---
        dram = ctx.enter_context(tc.tile_pool(name="dram", bufs=2, space="DRAM"))

        # --- constants / persistent ---
        ones_t = singles.tile([128, 1], F32R)
        nc.sync.dma_start(out=ones_t[:], in_=inp["ones"][:])
        eps_t = singles.tile([1, 1], F32)
        nc.vector.memset(eps_t[:], 1e-5)
        m01t = singles.tile([128, NSLOT, BLK], BF16)
        nc.sync.dma_start(out=m01t[:], in_=inp["m01"][:])

        xt = xp.tile([128, KD, TPC], F32R, tag="x")
        nc.sync.dma_start(out=xt[:], in_=inp["x0"][:])

        def ln_alloc(nm):
            s1 = ps_st.tile([1, TPC], F32, tag="st1", name=f"s1{nm}")
            s2 = ps_st.tile([1, TPC], F32, tag="st2", name=f"s2{nm}")
            return s1, s2

        def ln_stats_k(src, s1, s2, k):
            nc.tensor.matmul(s1[:], ones_t[:], src[:, k, :],
                             start=(k == 0), stop=(k == KD - 1))
            sq = tmp.tile([128, TPC], F32R, tag="tmp", name=f"sq_{k}")
            nc.vector.tensor_tensor(
                out=sq[:], in0=src[:, k, :].bitcast(F32),
                in1=src[:, k, :].bitcast(F32), op=ALU.mult)
            nc.tensor.matmul(s2[:], ones_t[:], sq[:],
                             start=(k == 0), stop=(k == KD - 1))

        def mean_chain(s1, nm):
            """s1 -> (m [1,TPC] f32, mB_s [128,TPC] f32 in SBUF)."""
            m = stat.tile([1, TPC], F32, tag="sa", name=f"m{nm}")
            nc.vector.tensor_scalar_mul(m[:], s1[:], 1.0 / D)
            mB_s = bcp.tile([128, TPC], F32, tag="mb", name=f"mb{nm}")
            nc.gpsimd.partition_broadcast(mB_s[:], m[:])
            return m, mB_s

        def xtilde(mB_s, nm):
            """x~ = x - mean, cast to bf16 (one op per strip)."""
            xtl = npool.tile([128, KD, TPC], BF16, tag="n", name=f"xt{nm}")
            for k in range(KD):
                nc.vector.tensor_tensor(
                    out=xtl[:, k, :], in0=xt[:, k, :].bitcast(F32),
                    in1=mB_s[:], op=ALU.subtract)
            return xtl

        def rs_chain(s2, m, nm, want_col=False):
            """-> rs_bs [128,TPC] f32 in SBUF (and rs_col [128,2] if asked)."""
            msq = stat.tile([1, TPC], F32, tag="sb", name=f"msq{nm}")
            nc.vector.tensor_tensor(out=msq[:], in0=m[:].bitcast(F32),
                                    in1=m[:].bitcast(F32), op=ALU.mult)
            var = stat.tile([1, TPC], F32, tag="sb", name=f"var{nm}")
            nc.vector.scalar_tensor_tensor(
                out=var[:], in0=s2[:], scalar=1.0 / D, in1=msq[:],
                op0=ALU.mult, op1=ALU.subtract)
            var2 = stat.tile([1, TPC], F32R, tag="sb", name=f"var2{nm}")
            nc.vector.tensor_scalar_add(var2[:], var[:], 1e-5)
            vr = stat.tile([1, TPC], F32, tag="sb", name=f"vr{nm}")
            nc.vector.reciprocal_approx_fast(vr[:], var2[:].bitcast(F32))
            rs = stat.tile([1, TPC], F32, tag="sb", name=f"rs{nm}")
            nc.scalar.activation(rs[:], vr[:], AF.Sqrt)
            rs_bs = bcp.tile([128, TPC], F32, tag="rs", name=f"rsb{nm}")
            nc.gpsimd.partition_broadcast(rs_bs[:], rs[:])
            return rs_bs

        def k_stage_gather(l, xb, mB_s, rs_bs, nks_t):
            """K = (wk^T xb - ksum*m) * rs -> staging -> AllGather.

            The matmuls contract the raw (uncentered) bf16 residual xb,
            so they queue immediately behind the previous FFN with no
            LN-stats dependency; the mean/scale fixup rides the PSUM
            eviction.  Staging DMAs go out per strip."""
            stage = stg.tile([128, KCOLS], BF16, tag="stgk", name=f"stagek{l}")
            ag_in = dram.tile([128, KCOLS], BF16, tag="agik", name=f"agik{l}")
            ag_out = dram.tile([G, 128, KCOLS], BF16, tag="agok",
                               name=f"agok{l}")
            # issue all wk loads upfront on the (idle) scalar HWDGE queue so
            # they bypass the sync-queue backlog of w2 prefetches
            wkts = []
            for m in range(KD):
                wkt = wq_p.tile([128, KD, 128], BF16, tag="wqk",
                                name=f"wk{l}_{m}")
                nc.scalar.dma_start(out=wkt[:], in_=inp[_f("wk", l)][:, m])
                wkts.append(wkt)
            for mp in range(KD // 2):
                pk = ps_mm.tile([128, 2, TPC], F32, tag="mm",
                                name=f"pk{l}_{mp}")
                for j in range(2):
                    m = 2 * mp + j
                    wkt = wkts[m]
                    for k in range(KD):
                        nc.tensor.matmul(pk[:, j, :], wkt[:, k, :],
                                         xb[:, k, :],
                                         start=(k == 0), stop=(k == KD - 1))
                    t = tmp.tile([128, TPC], F32, tag="tmp",
                                 name=f"kt{l}_{m}")
                    nc.vector.scalar_tensor_tensor(
                        out=t[:], in0=mB_s[:], scalar=nks_t[:, m:m + 1],
                        in1=pk[:, j, :], op0=ALU.mult, op1=ALU.add)
                    nc.vector.tensor_tensor(
                        out=stage[:, ts(m, TPC)], in0=t[:],
                        in1=rs_bs[:], op=ALU.mult)
                    nc.sync.dma_start(out=ag_in[:, ts(m, TPC)],
                                      in_=stage[:, ts(m, TPC)])
            nc.gpsimd.collective_compute(
                "AllGather", ALU.bypass, replica_groups=REPLICA_GROUPS,
                ins=[ag_in.opt()], outs=[ag_out.opt()])
            return ag_out

        def v_stage_gather(l, n1v):
            """V (token-major, + ones cols) -> staging -> AllGather."""
            stage = stg.tile([128, 2 * VSEG], BF16, tag="stgv",
                             name=f"stagev{l}")
            ones_view = stage[:].rearrange(
                "p (b h c) -> p b h c", b=2, h=H)[:, :, :, HD:]
            nc.vector.memset(ones_view, 1.0)
            for hh in range(2):   # halves of the head dim (512 cols)
                pva = ps_po.tile([128, 512], F32, tag="po",
                                 name=f"pva{l}_{hh}")
                pvb = ps_po.tile([128, 512], F32, tag="po",
                                 name=f"pvb{l}_{hh}")
                for k in range(KD):
                    wvs = wv_p.tile([128, 512], BF16, tag="wv",
                                    name=f"wv{l}_{hh}_{k}")
                    nc.sync.dma_start(out=wvs[:],
                                      in_=inp[_f("wv", l)][:, k, ts(hh, 512)])
                    nc.tensor.matmul(pva[:], n1v[:, k, ts(0, BLK)], wvs[:],
                                     start=(k == 0), stop=(k == KD - 1))
                    nc.tensor.matmul(pvb[:], n1v[:, k, ts(1, BLK)], wvs[:],
                                     start=(k == 0), stop=(k == KD - 1))
                for blk, pv in ((0, pva), (1, pvb)):
                    for j in range(8):
                        h = hh * 8 + j
                        nc.vector.tensor_scalar_mul(
                            stage[:, blk * VSEG + h * (HD + 1):
                                  blk * VSEG + h * (HD + 1) + HD],
                            pv[:, ts(j, HD)], 1.0)
            ag_in = dram.tile([128, 2 * VSEG], BF16, tag="agiv",
                              name=f"agiv{l}")
            ag_out = dram.tile([G, 128, 2 * VSEG], BF16, tag="agov",
                               name=f"agov{l}")
            nc.sync.dma_start(out=ag_in[:], in_=stage[:])
            nc.gpsimd.collective_compute(
                "AllGather", ALU.bypass, replica_groups=REPLICA_GROUPS,
                ins=[ag_in.opt()], outs=[ag_out.opt()])
            return ag_out

        def q_proj(l, xtl, rs_bs):
            q = qp.tile([128, KD, TPC], BF16, tag="q", name=f"q{l}")
            for mp in range(KD // 2):
                pq = ps_mm.tile([128, 2, TPC], F32, tag="mm",
                                name=f"pq{l}_{mp}")
                for j in range(2):
                    m = 2 * mp + j
                    wqt = wq_p.tile([128, KD, 128], BF16, tag="wqk",
                                    name=f"wq{l}_{m}")
                    nc.sync.dma_start(out=wqt[:], in_=inp[_f("wq", l)][:, m])
                    for k in range(KD):
                        nc.tensor.matmul(pq[:, j, :], wqt[:, k, :],
                                         xtl[:, k, :],
                                         start=(k == 0), stop=(k == KD - 1))
                    nc.vector.tensor_tensor(
                        out=q[:, m, :], in0=pq[:, j, :], in1=rs_bs[:],
                        op=ALU.mult)
            return q

        def attention(l, q, kg, vg, oT):

            def phase1(h):
                pp = 64 * (h % 2)
                hc = h // 2
                eTile = ep.tile([128, NSLOT, BLK], BF16, tag="eT",
                                name=f"eT{l}_{h}")
                # scores (transposed [k, q]) + exp + causal mask,
                # batched 4 slots per PSUM bank
                for grp in range(NSLOT // 4):
                    pss = ps_mm.tile([128, 4, BLK], F32, tag="mm",
                                     name=f"sc{l}_{h}_{grp}")
                    for j in range(4):
                        s = 4 * grp + j
                        kb = s if s < 4 else s - 4
                        qc = ts(0, BLK) if s < 4 else ts(1, BLK)
                        rho, sg = RHO[kb], SIG[kb]
                        nc.tensor.matmul(
                            pss[:, j, :],
                            kg[pp:pp + 64, rho,
                               hc * TPC + sg * BLK: hc * TPC + sg * BLK + BLK],
                            q[pp:pp + 64, hc, qc], start=True, stop=True)
                    nc.scalar.activation(eTile[:, ts(grp, 4), :], pss[:],
                                         AF.Exp)
                    nc.vector.tensor_tensor(
                        out=eTile[:, ts(grp, 4), :],
                        in0=eTile[:, ts(grp, 4), :],
                        in1=m01t[:, ts(grp, 4), :], op=ALU.mult)
                return eTile

            def phase2(h, eTile):
                pp = 64 * (h % 2)
                hc = h // 2
                # AV (+ ones-column denominator); A-side (cols 0:128) and
                # B-side (cols 128:256) are two accumulation groups in ONE
                # bank — the B start=True clears only has_written bits, the
                # already-final A values persist.
                pso = ps_po.tile([HD + 1, TPC], F32, tag="po",
                                 name=f"pav{l}_{h}")
                for s in range(NSLOT):
                    kb = s if s < 4 else s - 4
                    rho, sg = RHO[kb], SIG[kb]
                    cc = ts(0, BLK) if s < 4 else ts(1, BLK)
                    vsl = vg[:, rho,
                             sg * VSEG + h * (HD + 1):
                             sg * VSEG + (h + 1) * (HD + 1)]
                    nc.tensor.matmul(pso[:, cc], vsl, eTile[:, s, :],
                                     start=(s in (0, 4)),
                                     stop=(s in (3, NSLOT - 1)),
                                     skip_group_check=True)
                # free the PSUM slot fast: copy den row + raw numerators,
                # then normalize off-PSUM with the 1-pass approx reciprocal
                dh = stat.tile([1, TPC], F32, tag="rr", name=f"dh{l}_{h}")
                nc.vector.tensor_scalar_mul(dh[:], pso[HD:HD + 1, :], 1.0)
                nc.vector.tensor_scalar_mul(
                    oT[pp:pp + 64, hc, :], pso[0:HD, :], 1.0)
                rh = stat.tile([1, TPC], F32, tag="rr", name=f"rh{l}_{h}")
                nc.vector.reciprocal_approx_fast(rh[:], dh[:])
                bb = bcp.tile([128, TPC], F32, tag="bb", name=f"bb{l}_{h}")
                nc.gpsimd.partition_broadcast(bb[:], rh[:])
                nc.vector.tensor_tensor(
                    out=oT[pp:pp + 64, hc, :], in0=oT[pp:pp + 64, hc, :],
                    in1=bb[pp:pp + 64, :], op=ALU.mult)

            held = [phase1(h) for h in range(H)]
            for h in range(H):
                phase2(h, held[h])

        def wo_residual(l, oT, s1, s2):
            for mp in range(KD // 2):
                po = ps_mm.tile([128, 2, TPC], F32, tag="mm",
                                name=f"po{l}_{mp}")
                for j in range(2):
                    m = 2 * mp + j
                    wot = wq_p.tile([128, KD, 128], BF16, tag="wqk",
                                    name=f"wo{l}_{m}")
                    nc.sync.dma_start(out=wot[:], in_=inp[_f("wo", l)][:, m])
                    for k in range(KD):
                        nc.tensor.matmul(po[:, j, :], wot[:, k, :],
                                         oT[:, k, :],
                                         start=(k == 0), stop=(k == KD - 1))
                    nc.vector.tensor_tensor(
                        out=xt[:, m, :], in0=xt[:, m, :].bitcast(F32),
                        in1=po[:, j, :], op=ALU.add)
                for j in range(2):
                    ln_stats_k(xt, s1, s2, 2 * mp + j)

        def ffn(l, xtl2, rs_bs2, s1, s2, xb_next):
            b1_t = bias.tile([128, M1], F32, tag="bias", name=f"b1{l}")
            nc.sync.dma_start(out=b1_t[:], in_=inp[_f("b1", l)][:])
            b2_t = bias.tile([128, M2], F32, tag="bias", name=f"b2{l}")
            nc.sync.dma_start(out=b2_t[:], in_=inp[_f("b2", l)][:])
            hT = hp.tile([128, M1, TPC], BF16, tag="h", name=f"hT{l}")
            for mp in range(M1 // 2):
                p1 = ps_mm.tile([128, 2, TPC], F32, tag="mm",
                                name=f"p1{l}_{mp}")
                for j in range(2):
                    m = 2 * mp + j
                    w1t = wq_p.tile([128, KD, 128], BF16, tag="wqk",
                                    name=f"w1{l}_{m}")
                    nc.sync.dma_start(out=w1t[:], in_=inp[_f("w1", l)][:, m])
                    for k in range(KD):
                        nc.tensor.matmul(p1[:, j, :], w1t[:, k, :],
                                         xtl2[:, k, :],
                                         start=(k == 0), stop=(k == KD - 1))
                    u = tmp.tile([128, TPC], F32, tag="tmp",
                                 name=f"u{l}_{m}")
                    nc.vector.tensor_tensor(
                        out=u[:], in0=p1[:, j, :], in1=rs_bs2[:],
                        op=ALU.mult)
                    nc.scalar.activation(hT[:, m, :], u[:], AF.Gelu,
                                         bias=b1_t[:, m:m + 1])
            for mp in range(M2 // 2):
                p2 = ps_mm.tile([128, 2, TPC], F32, tag="mm",
                                name=f"p2{l}_{mp}")
                for j in range(2):
                    m = 2 * mp + j
                    w2t = w2_p.tile([128, M1, 128], BF16, tag="w2",
                                    name=f"w2{l}_{m}")
                    nc.sync.dma_start(out=w2t[:], in_=inp[_f("w2", l)][:, m])
                    for k in range(M1):
                        nc.tensor.matmul(p2[:, j, :], w2t[:, k, :],
                                         hT[:, k, :],
                                         start=(k == 0), stop=(k == M1 - 1))
                    nc.vector.scalar_tensor_tensor(
                        out=xt[:, m, :], in0=p2[:, j, :],
                        scalar=b2_t[:, m:m + 1],
                        in1=xt[:, m, :].bitcast(F32), op0=ALU.add,
                        op1=ALU.add)
                    nc.scalar.copy(xb_next[:, m, :],
                                   xt[:, m, :].bitcast(F32))
                for j in range(2):
                    ln_stats_k(xt, s1, s2, 2 * mp + j)

        def tap_f32(name):
            if name in taps:
                nc.sync.dma_start(out=taps[name][:], in_=xt[:].bitcast(F32))

        # ---------------- main loop ----------------
        s1, s2 = ln_alloc("ln_0")
        xb = npool.tile([128, KD, TPC], BF16, tag="n", name="xb0")
        for k in range(KD):
            ln_stats_k(xt, s1, s2, k)
            nc.scalar.copy(xb[:, k, :], xt[:, k, :].bitcast(F32))
        for l in range(L):
            nks_t = bias.tile([128, KD], F32, tag="nks", name=f"nks{l}")
            nc.sync.dma_start(out=nks_t[:], in_=inp[_f("nks", l)][:])
            m, mB_s = mean_chain(s1, f"l{l}")
            rs_bs = rs_chain(s2, m, f"l{l}")
            agk = k_stage_gather(l, xb, mB_s, rs_bs, nks_t)
            xtl = xtilde(mB_s, f"l{l}")
            n1v = npool.tile([128, KD, TPC], BF16, tag="n", name=f"n1v{l}")
            for k in range(KD):
                nc.vector.tensor_tensor(
                    out=n1v[:, k, :], in0=xtl[:, k, :], in1=rs_bs[:],
                    op=ALU.mult)
            agv = v_stage_gather(l, n1v)
            q = q_proj(l, xtl, rs_bs)
            kg = kvp.tile([128, G, KCOLS], BF16, tag="kg", name=f"kg{l}")
            for rho in range(G):
                nc.sync.dma_start(out=kg[:, rho, :], in_=agk[rho])
            vg = kvp.tile([128, G, 2 * VSEG], BF16, tag="vg", name=f"vg{l}")
            for rho in range(G):
                nc.sync.dma_start(out=vg[:, rho, :], in_=agv[rho])
            oT = op.tile([128, KD, TPC], BF16, tag="oT", name=f"oT{l}")
            attention(l, q, kg, vg, oT)
            s1a, s2a = ln_alloc(f"ln2_{l}")
            wo_residual(l, oT, s1a, s2a)
            if l == 0:
                tap_f32("xa_0")
            m2, m2B_s = mean_chain(s1a, f"f{l}")
            xtl2 = xtilde(m2B_s, f"f{l}")
            rs_bs2 = rs_chain(s2a, m2, f"f{l}")
            s1, s2 = ln_alloc(f"ln1_{l + 1}")
            xb = npool.tile([128, KD, TPC], BF16, tag="n", name=f"xb{l + 1}")
            ffn(l, xtl2, rs_bs2, s1, s2, xb)
            if l == 0:
                tap_f32("x_1")

        # final LN -> nf
        mf, mfB_s = mean_chain(s1, "fin")
        xtlf = xtilde(mfB_s, "fin")
        rs_bsf = rs_chain(s2, mf, "fin")
        nf = npool.tile([128, KD, TPC], BF16, tag="n", name="nf")
        for k in range(KD):
            nc.vector.tensor_tensor(
                out=nf[:, k, :], in0=xtlf[:, k, :], in1=rs_bsf[:],
                op=ALU.mult)
        if "nf" in taps:
            f = stg.tile([128, KD, TPC], F32, tag="tapf", name="tpnf")
            nc.scalar.copy(f[:], nf[:])
            nc.sync.dma_start(out=taps["nf"][:], in_=f[:])

        # final AllGather of nf, then vocab-sharded logits
        nf_in = dram.tile([128, NFCOLS], BF16, tag="nfi")
        nf_out = dram.tile([G, 128, NFCOLS], BF16, tag="nfo")
        nc.sync.dma_start(out=nf_in[:], in_=nf[:])
        nc.gpsimd.collective_compute(
            "AllGather", ALU.bypass, replica_groups=REPLICA_GROUPS,
            ins=[nf_in.opt()], outs=[nf_out.opt()])
        nfg = hp.tile([128, G, KD, TPC], BF16, tag="h", name="nfg")
        for rho in range(G):
            nc.sync.dma_start(out=nfg[:, rho], in_=nf_out[rho])

        for vb in range(NVB):
            ebt = embp.tile([128, KD, 128], BF16, tag="emb", name=f"eb{vb}")
            nc.sync.dma_start(out=ebt[:], in_=inp["emb"][:, vb])
            for half in range(2):
                pl = ps_mm.tile([128, 512], F32, tag="mm",
                                name=f"pl{vb}_{half}")
                for k in range(KD):
                    nc.tensor.matmul(pl[:], ebt[:, k, :],
                                     nfg[:, ts(half, 2), k, :],
                                     start=(k == 0), stop=(k == KD - 1))
                lo = lout.tile([128, 512], BF16, tag="lo",
                               name=f"lo{vb}_{half}")
                if (vb + half) % 2 == 0:
                    nc.scalar.copy(lo[:], pl[:])
                else:
                    nc.vector.tensor_scalar_mul(lo[:], pl[:], 1.0)
                nc.sync.dma_start(out=logits[ts(vb, 128), ts(half, 512)],
                                  in_=lo[:])


# ------------------------------------------------------------------
# Host side
# ------------------------------------------------------------------

def _kfold(w):
    """[in, out] -> [128, in//128, out]."""
    i, o = w.shape
    return np.ascontiguousarray(w.reshape(i // 128, 128, o).transpose(1, 0, 2))


def _mslice(w):
    """[in, out] -> [128, out//128, in//128, 128] contiguous strips."""
    i, o = w.shape
    t = w.reshape(i // 128, 128, o // 128, 128)
    return np.ascontiguousarray(t.transpose(1, 2, 0, 3))


def _cols(v):
    """[n] -> [128, n//128] per-partition bias columns."""
    return np.ascontiguousarray(v.reshape(-1, 128).T)


def _bf(a):
    return np.ascontiguousarray(a).astype(BF16NP)


def prep_inputs(inputs):
    f = lambda a: np.asarray(a, np.float32)
    tokens = np.asarray(inputs["tokens"])
    tok_emb, pos_emb = f(inputs["tok_emb"]), f(inputs["pos_emb"])
    ln1_g = f(inputs["ln1_g"])
    wq, wk = f(inputs["wq"]), f(inputs["wk"])
    wv, wo = f(inputs["wv"]), f(inputs["wo"])
    ln2_g, ln2_b = f(inputs["ln2_g"]), f(inputs["ln2_b"])
    w1, b1 = f(inputs["w1"]), f(inputs["b1"])
    w2, b2 = f(inputs["w2"]), f(inputs["b2"])
    lnf_g = f(inputs["lnf_g"])

    sc = 1.0 / np.sqrt(HD)
    x0 = tok_emb[tokens] + pos_emb[:S][None]          # [B, S, D]
    ones = np.ones((128, 1), np.float32)

    # shared (identical on all cores) weight tensors
    shared = {"ones": ones}
    for l in range(L):
        shared[_f("wq", l)] = _bf(_mslice(ln1_g[l][:, None] * wq[l] * sc))
        shared[_f("wk", l)] = _bf(_mslice(ln1_g[l][:, None] * wk[l]))
        shared[_f("nks", l)] = _cols(-np.asarray(
            _bf(ln1_g[l][:, None] * wk[l]), np.float32).sum(0))
        shared[_f("wv", l)] = _bf(_kfold(ln1_g[l][:, None] * wv[l]))
        shared[_f("wo", l)] = _bf(_mslice(wo[l]))
        shared[_f("w1", l)] = _bf(_mslice(ln2_g[l][:, None] * w1[l]))
        shared[_f("w2", l)] = _bf(_mslice(w2[l]))
        shared[_f("b1", l)] = _cols(b1[l] + ln2_b[l] @ w1[l])
        shared[_f("b2", l)] = _cols(b2[l])

    in_maps = []
    for core in range(N_CORES):
        g, r = core // G, core % G
        A_blk, B_blk = r, 7 - r
        m = dict(shared)
        xo = np.concatenate([x0[g, 128 * A_blk:128 * A_blk + 128],
                             x0[g, 128 * B_blk:128 * B_blk + 128]], 0)
        m["x0"] = _kfold(np.ascontiguousarray(xo.T))
        m01 = np.zeros((128, NSLOT, BLK), np.float32)
        kp = np.arange(128)[:, None]
        qf = np.arange(128)[None, :]
        for s in range(NSLOT):
            qb = A_blk if s < 4 else B_blk
            kb = s if s < 4 else s - 4
            m01[:, s, :] = (128 * kb + kp <= 128 * qb + qf)
        m["m01"] = _bf(m01)
        v0 = r * VS
        v1 = min(v0 + VS, V)
        epad = np.zeros((D, VSP), np.float32)
        epad[:, :v1 - v0] = (tok_emb[v0:v1] * lnf_g[None, :]).T
        m["emb"] = _bf(_mslice(epad))
        in_maps.append(m)
    return in_maps


_CACHED = {}


def _get_program(debug_taps=False):
    key = bool(debug_taps)
    if key not in _CACHED:
        _CACHED[key] = build_program(debug_taps)
    return _CACHED[key]


def run(inputs, debug_taps=False, trace=False, **kw):
    nc = _get_program(debug_taps)
    in_maps = prep_inputs(inputs)
    return run_bass_kernel_spmd(nc, in_maps, list(range(N_CORES)),
                                trace=trace, **kw)


# token column -> natural token index within a group's 1024 tokens
def _colperm():
    perm = np.empty(T, np.int64)
    for c in range(T):
        rho, rem = divmod(c, 256)
        half, qf = divmod(rem, 128)
        blkid = rho if half == 0 else 7 - rho
        perm[c] = 128 * blkid + qf
    return perm


def assemble(results, inputs):
    lnf_b = np.asarray(inputs["lnf_b"], np.float32)
    tok_emb = np.asarray(inputs["tok_emb"], np.float32)
    perm = _colperm()
    out = np.empty((B, S, V), np.float32)
    for b in range(B):
        for r in range(G):
            v0 = r * VS
            v1 = min(v0 + VS, V)
            part = results[b * G + r]["logits"][:v1 - v0, :]  # [rows, T]
            out[b, perm, v0:v1] = part.T.astype(np.float32)
    if np.any(lnf_b):
        out += (tok_emb @ lnf_b)[None, None, :]
    return out


def kernel(**inputs):
    res = run(inputs)
    return assemble(res.results, inputs)


if __name__ == "__main__":
    print("building program...")
    build_program()
    print("build + compile OK")


# revision 43
# speedup vs baseline: 1.0848x; 1.0009x over previous
"""GPT forward pass on 8 Trainium2 NeuronCores — sequence-parallel (SP8).

Model: B=2, S=1024, D=1024, H=16 heads (hd=64), L=6 layers, V=50257,
tied embedding head.

Sharding: the 2048 tokens are split into 16 causal blocks of 128;
core c (group g=c//4 over batch, rank r=c%4) owns query blocks
A=r and B=7-r of batch g (256 tokens), which balances causal attention
work.  Every core holds the FULL weights (bf16); the only per-layer
communication is a bf16 AllGather of K then V (~0.5 MB each) within
each 4-core group.  The final LN output is AllGathered once before the
vocab-sharded tied-logit matmul.

Perf structure (v2):
- Deferred LayerNorm: projections contract x~ = x - mean directly
  (built with one vector op per strip); the 1/sigma scale is folded
  into the PSUM evictions.  This removes the serial LN-finish chain
  between FFN and the next layer's QKV projections, so the K/V
  AllGather triggers earlier.
- Attention holds all 16 heads' exp(scores) tiles so the serialized
  V AllGather hides behind phase-1 (scores+exp) work.
- Softmax normalization is batched: denominators are copied into one
  row, one reciprocal_approx_fast over [1, 4096], broadcast back via
  tiny ones-matmuls, and applied with 8 vector ops (instead of 32
  slow [1,128] reciprocals + gpsimd broadcasts).
- Logits are emitted in bf16 (halves the output DMA) and upcast on
  the host.
"""

import sys

sys.path.insert(0, "/opt/trn_rl_repo")

import contextlib

import numpy as np
import ml_dtypes

import concourse.bacc as bacc
import concourse.mybir as mybir
import concourse.tile as tile
from concourse.bass import ts
from concourse.bass_utils import run_bass_kernel_spmd

F32 = mybir.dt.float32
F32R = mybir.dt.float32r
BF16 = mybir.dt.bfloat16
AF = mybir.ActivationFunctionType
ALU = mybir.AluOpType
BF16NP = ml_dtypes.bfloat16

# Model dims
B, S, D, H, L, V = 2, 1024, 1024, 16, 6, 50257
HD = D // H            # 64
DFF = 4 * D            # 4096
N_CORES = 8
G = 4                  # group size (cores per batch element)
KD = D // 128          # 8 feature tiles
HC = H // 2            # 8 head-chunks (2 heads per 128 partitions)
TPC = 256              # tokens per core
BLK = 128              # token block
M1 = DFF // 128        # 32 w1 out strips
M2 = KD                # 8 w2 out strips
NSLOT = 12             # attention slots per head (4 A-side + 8 B-side)
VS = 12565             # vocab rows per group-rank (last: 12562)
VSP = 12800            # padded
NVB = VSP // 128       # 100 vocab blocks
T = 1024               # tokens per group (gathered)

KCOLS = HC * TPC            # 2048 k cols in kv contribution
VSEG = H * (HD + 1)         # 1040 v cols per token block (ones col incl.)
NFCOLS = KD * TPC           # 2048

# key block kb -> (rank, slot-within-rank)
RHO = [kb if kb < 4 else 7 - kb for kb in range(8)]
SIG = [0 if kb < 4 else 1 for kb in range(8)]

REPLICA_GROUPS = [[0, 1, 2, 3], [4, 5, 6, 7]]


def _f(name, l=None):
    return name if l is None else f"{name}{l}"


def build_program(debug_taps=False):
    nc = bacc.Bacc("TRN2", target_bir_lowering=False, debug=False,
                   enable_asserts=True, num_devices=N_CORES)

    inp = {}

    def dram_in(name, shape, dtype=BF16):
        inp[name] = nc.dram_tensor(name, shape, dtype, kind="ExternalInput").ap()
        return inp[name]

    dram_in("x0", [128, KD, TPC], F32R)
    dram_in("ones", [128, 1], F32R)
    dram_in("m01", [128, NSLOT, BLK], BF16)
    for l in range(L):
        dram_in(_f("wq", l), [128, KD, KD, 128])    # [p, m, kt, 128]
        dram_in(_f("wk", l), [128, KD, KD, 128])
        dram_in(_f("wv", l), [128, KD, D])          # plain k-fold (moving)
        dram_in(_f("wo", l), [128, KD, KD, 128])
        dram_in(_f("w1", l), [128, M1, KD, 128])
        dram_in(_f("w2", l), [128, M2, M1, 128])
        dram_in(_f("b1", l), [128, M1], F32)
        dram_in(_f("b2", l), [128, M2], F32)
        dram_in(_f("nks", l), [128, KD], F32)       # -colsum(wk), per strip
    dram_in("emb", [128, NVB, KD, 128])
    logits = nc.dram_tensor("logits", [VSP, T], BF16, kind="ExternalOutput").ap()

    taps = {}
    if debug_taps:
        for name in ["xa_0", "x_1", "nf"]:
            taps[name] = nc.dram_tensor("dbg_" + name, [128, KD, TPC], F32,
                                        kind="ExternalOutput").ap()

    with tile.TileContext(nc) as tc:
        _body(tc, inp, logits, taps)
    nc.compile()
    return nc


def _body(tc, inp, logits, taps):
    nc = tc.nc
    ctx = contextlib.ExitStack()
    with ctx:
        # --- SBUF pools ---
        singles = ctx.enter_context(tc.tile_pool(name="singles", bufs=1))
        xp = ctx.enter_context(tc.tile_pool(name="xp", bufs=1))        # 8K
        npool = ctx.enter_context(tc.tile_pool(name="npool", bufs=2))  # 8K
        tmp = ctx.enter_context(tc.tile_pool(name="tmp", bufs=3))      # 3K
        qp = ctx.enter_context(tc.tile_pool(name="qp", bufs=1))        # 4K
        stg = ctx.enter_context(tc.tile_pool(name="stg", bufs=1))      # 8.25K
        kvp = ctx.enter_context(tc.tile_pool(name="kvp", bufs=1))      # 33K
        op = ctx.enter_context(tc.tile_pool(name="op", bufs=1))        # 4K
        hp = ctx.enter_context(tc.tile_pool(name="hp", bufs=1))        # 16K
        wq_p = ctx.enter_context(tc.tile_pool(name="wq_p", bufs=8))    # 16K
        wv_p = ctx.enter_context(tc.tile_pool(name="wv_p", bufs=4))    # 4K
        w2_p = ctx.enter_context(tc.tile_pool(name="w2_p", bufs=3))    # 24K
        embp = ctx.enter_context(tc.tile_pool(name="embp", bufs=2))    # 4K
        ep = ctx.enter_context(tc.tile_pool(name="ep", bufs=16))       # 48K
        stat = ctx.enter_context(tc.tile_pool(name="stat", bufs=2))
        bcp = ctx.enter_context(tc.tile_pool(name="bcp", bufs=2))      # 2K
        lout = ctx.enter_context(tc.tile_pool(name="lout", bufs=2))    # 2K
        bias = ctx.enter_context(tc.tile_pool(name="bias", bufs=2))
        # --- PSUM pools (8 banks) ---
        ps_mm = ctx.enter_context(tc.tile_pool(name="ps_mm", bufs=4, space="PSUM"))
        ps_po = ctx.enter_context(tc.tile_pool(name="ps_po", bufs=2, space="PSUM"))
        ps_st = ctx.enter_context(tc.tile_pool(name="ps_st", bufs=1, space="PSUM"))
        # --- DRAM (collective bounce) ---
        dram = ctx.enter_context(tc.tile_pool(name="dram", bufs=2, space="DRAM"))

        # --- constants / persistent ---
        ones_t = singles.tile([128, 1], F32R)
        nc.sync.dma_start(out=ones_t[:], in_=inp["ones"][:])
        eps_t = singles.tile([1, 1], F32)
        nc.vector.memset(eps_t[:], 1e-5)
        m01t = singles.tile([128, NSLOT, BLK], BF16)
        nc.sync.dma_start(out=m01t[:], in_=inp["m01"][:])

        xt = xp.tile([128, KD, TPC], F32R, tag="x")
        nc.sync.dma_start(out=xt[:], in_=inp["x0"][:])

        def ln_alloc(nm):
            s1 = ps_st.tile([1, TPC], F32, tag="st1", name=f"s1{nm}")
            s2 = ps_st.tile([1, TPC], F32, tag="st2", name=f"s2{nm}")
            return s1, s2

        def ln_stats_k(src, s1, s2, k):
            nc.tensor.matmul(s1[:], ones_t[:], src[:, k, :],
                             start=(k == 0), stop=(k == KD - 1))
            sq = tmp.tile([128, TPC], F32R, tag="tmp", name=f"sq_{k}")
            nc.vector.tensor_tensor(
                out=sq[:], in0=src[:, k, :].bitcast(F32),
                in1=src[:, k, :].bitcast(F32), op=ALU.mult)
            nc.tensor.matmul(s2[:], ones_t[:], sq[:],
                             start=(k == 0), stop=(k == KD - 1))

        def mean_chain(s1, nm):
            """s1 -> (m [1,TPC] f32, mB_s [128,TPC] f32 in SBUF)."""
            m = stat.tile([1, TPC], F32, tag="sa", name=f"m{nm}")
            nc.vector.tensor_scalar_mul(m[:], s1[:], 1.0 / D)
            mB_s = bcp.tile([128, TPC], F32, tag="mb", name=f"mb{nm}")
            nc.gpsimd.partition_broadcast(mB_s[:], m[:])
            return m, mB_s

        def xtilde(mB_s, nm):
            """x~ = x - mean, cast to bf16 (one op per strip)."""
            xtl = npool.tile([128, KD, TPC], BF16, tag="n", name=f"xt{nm}")
            for k in range(KD):
                nc.vector.tensor_tensor(
                    out=xtl[:, k, :], in0=xt[:, k, :].bitcast(F32),
                    in1=mB_s[:], op=ALU.subtract)
            return xtl

        def rs_chain(s2, m, nm, want_col=False):
            """-> rs_bs [128,TPC] f32 in SBUF (and rs_col [128,2] if asked)."""
            msq = stat.tile([1, TPC], F32, tag="sb", name=f"msq{nm}")
            nc.vector.tensor_tensor(out=msq[:], in0=m[:].bitcast(F32),
                                    in1=m[:].bitcast(F32), op=ALU.mult)
            var = stat.tile([1, TPC], F32, tag="sb", name=f"var{nm}")
            nc.vector.scalar_tensor_tensor(
                out=var[:], in0=s2[:], scalar=1.0 / D, in1=msq[:],
                op0=ALU.mult, op1=ALU.subtract)
            var2 = stat.tile([1, TPC], F32R, tag="sb", name=f"var2{nm}")
            nc.vector.tensor_scalar_add(var2[:], var[:], 1e-5)
            vr = stat.tile([1, TPC], F32, tag="sb", name=f"vr{nm}")
            nc.vector.reciprocal_approx_fast(vr[:], var2[:].bitcast(F32))
            rs = stat.tile([1, TPC], F32, tag="sb", name=f"rs{nm}")
            nc.scalar.activation(rs[:], vr[:], AF.Sqrt)
            rs_bs = bcp.tile([128, TPC], F32, tag="rs", name=f"rsb{nm}")
            nc.gpsimd.partition_broadcast(rs_bs[:], rs[:])
            return rs_bs

        def k_stage_gather(l, xb, mB_s, rs_bs, nks_t):
            """K = (wk^T xb - ksum*m) * rs -> staging -> AllGather.

            The matmuls contract the raw (uncentered) bf16 residual xb,
            so they queue immediately behind the previous FFN with no
            LN-stats dependency; the mean/scale fixup rides the PSUM
            eviction.  Staging DMAs go out per strip."""
            stage = stg.tile([128, KCOLS], BF16, tag="stgk", name=f"stagek{l}")
            ag_in = dram.tile([128, KCOLS], BF16, tag="agik", name=f"agik{l}")
            ag_out = dram.tile([G, 128, KCOLS], BF16, tag="agok",
                               name=f"agok{l}")
            # issue all wk loads upfront on the (idle) scalar HWDGE queue so
            # they bypass the sync-queue backlog of w2 prefetches
            wkts = []
            for m in range(KD):
                wkt = wq_p.tile([128, KD, 128], BF16, tag="wqk",
                                name=f"wk{l}_{m}")
                nc.scalar.dma_start(out=wkt[:], in_=inp[_f("wk", l)][:, m])
                wkts.append(wkt)
            for mp in range(KD // 2):
                pk = ps_mm.tile([128, 2, TPC], F32, tag="mm",
                                name=f"pk{l}_{mp}")
                for j in range(2):
                    m = 2 * mp + j
                    wkt = wkts[m]
                    for k in range(KD):
                        nc.tensor.matmul(pk[:, j, :], wkt[:, k, :],
                                         xb[:, k, :],
                                         start=(k == 0), stop=(k == KD - 1))
                    t = tmp.tile([128, TPC], F32, tag="tmp",
                                 name=f"kt{l}_{m}")
                    nc.vector.scalar_tensor_tensor(
                        out=t[:], in0=mB_s[:], scalar=nks_t[:, m:m + 1],
                        in1=pk[:, j, :], op0=ALU.mult, op1=ALU.add)
                    nc.vector.tensor_tensor(
                        out=stage[:, ts(m, TPC)], in0=t[:],
                        in1=rs_bs[:], op=ALU.mult)
                    nc.sync.dma_start(out=ag_in[:, ts(m, TPC)],
                                      in_=stage[:, ts(m, TPC)])
            nc.gpsimd.collective_compute(
                "AllGather", ALU.bypass, replica_groups=REPLICA_GROUPS,
                ins=[ag_in.opt()], outs=[ag_out.opt()])
            return ag_out

        def v_stage_gather(l, n1v):
            """V (token-major, + ones cols) -> staging -> AllGather."""
            stage = stg.tile([128, 2 * VSEG], BF16, tag="stgv",
                             name=f"stagev{l}")
            ones_view = stage[:].rearrange(
                "p (b h c) -> p b h c", b=2, h=H)[:, :, :, HD:]
            nc.vector.memset(ones_view, 1.0)
            for hh in range(2):   # halves of the head dim (512 cols)
                pva = ps_po.tile([128, 512], F32, tag="po",
                                 name=f"pva{l}_{hh}")
                pvb = ps_po.tile([128, 512], F32, tag="po",
                                 name=f"pvb{l}_{hh}")
                for k in range(KD):
                    wvs = wv_p.tile([128, 512], BF16, tag="wv",
                                    name=f"wv{l}_{hh}_{k}")
                    nc.sync.dma_start(out=wvs[:],
                                      in_=inp[_f("wv", l)][:, k, ts(hh, 512)])
                    nc.tensor.matmul(pva[:], n1v[:, k, ts(0, BLK)], wvs[:],
                                     start=(k == 0), stop=(k == KD - 1))
                    nc.tensor.matmul(pvb[:], n1v[:, k, ts(1, BLK)], wvs[:],
                                     start=(k == 0), stop=(k == KD - 1))
                for blk, pv in ((0, pva), (1, pvb)):
                    for j in range(8):
                        h = hh * 8 + j
                        nc.vector.tensor_scalar_mul(
                            stage[:, blk * VSEG + h * (HD + 1):
                                  blk * VSEG + h * (HD + 1) + HD],
                            pv[:, ts(j, HD)], 1.0)
            ag_in = dram.tile([128, 2 * VSEG], BF16, tag="agiv",
                              name=f"agiv{l}")
            ag_out = dram.tile([G, 128, 2 * VSEG], BF16, tag="agov",
                               name=f"agov{l}")
            nc.sync.dma_start(out=ag_in[:], in_=stage[:])
            nc.gpsimd.collective_compute(
                "AllGather", ALU.bypass, replica_groups=REPLICA_GROUPS,
                ins=[ag_in.opt()], outs=[ag_out.opt()])
            return ag_out

        def q_proj(l, xtl, rs_bs):
            q = qp.tile([128, KD, TPC], BF16, tag="q", name=f"q{l}")
            wqts = []
            for m in range(KD):
                wqt = wq_p.tile([128, KD, 128], BF16, tag="wqk",
                                name=f"wq{l}_{m}")
                nc.scalar.dma_start(out=wqt[:], in_=inp[_f("wq", l)][:, m])
                wqts.append(wqt)
            for mp in range(KD // 2):
                pq = ps_mm.tile([128, 2, TPC], F32, tag="mm",
                                name=f"pq{l}_{mp}")
                for j in range(2):
                    m = 2 * mp + j
                    wqt = wqts[m]
                    for k in range(KD):
                        nc.tensor.matmul(pq[:, j, :], wqt[:, k, :],
                                         xtl[:, k, :],
                                         start=(k == 0), stop=(k == KD - 1))
                    nc.vector.tensor_tensor(
                        out=q[:, m, :], in0=pq[:, j, :], in1=rs_bs[:],
                        op=ALU.mult)
            return q

        def attention(l, q, kg, vg, oT):

            def phase1(h):
                pp = 64 * (h % 2)
                hc = h // 2
                eTile = ep.tile([128, NSLOT, BLK], BF16, tag="eT",
                                name=f"eT{l}_{h}")
                # scores (transposed [k, q]) + exp + causal mask,
                # batched 4 slots per PSUM bank
                for grp in range(NSLOT // 4):
                    pss = ps_mm.tile([128, 4, BLK], F32, tag="mm",
                                     name=f"sc{l}_{h}_{grp}")
                    for j in range(4):
                        s = 4 * grp + j
                        kb = s if s < 4 else s - 4
                        qc = ts(0, BLK) if s < 4 else ts(1, BLK)
                        rho, sg = RHO[kb], SIG[kb]
                        nc.tensor.matmul(
                            pss[:, j, :],
                            kg[pp:pp + 64, rho,
                               hc * TPC + sg * BLK: hc * TPC + sg * BLK + BLK],
                            q[pp:pp + 64, hc, qc], start=True, stop=True)
                    nc.scalar.activation(eTile[:, ts(grp, 4), :], pss[:],
                                         AF.Exp)
                    nc.vector.tensor_tensor(
                        out=eTile[:, ts(grp, 4), :],
                        in0=eTile[:, ts(grp, 4), :],
                        in1=m01t[:, ts(grp, 4), :], op=ALU.mult)
                return eTile

            def phase2(h, eTile):
                pp = 64 * (h % 2)
                hc = h // 2
                # AV (+ ones-column denominator); A-side (cols 0:128) and
                # B-side (cols 128:256) are two accumulation groups in ONE
                # bank — the B start=True clears only has_written bits, the
                # already-final A values persist.
                pso = ps_po.tile([HD + 1, TPC], F32, tag="po",
                                 name=f"pav{l}_{h}")
                for s in range(NSLOT):
                    kb = s if s < 4 else s - 4
                    rho, sg = RHO[kb], SIG[kb]
                    cc = ts(0, BLK) if s < 4 else ts(1, BLK)
                    vsl = vg[:, rho,
                             sg * VSEG + h * (HD + 1):
                             sg * VSEG + (h + 1) * (HD + 1)]
                    nc.tensor.matmul(pso[:, cc], vsl, eTile[:, s, :],
                                     start=(s in (0, 4)),
                                     stop=(s in (3, NSLOT - 1)),
                                     skip_group_check=True)
                # free the PSUM slot fast: copy den row + raw numerators,
                # then normalize off-PSUM with the 1-pass approx reciprocal
                dh = stat.tile([1, TPC], F32, tag="rr", name=f"dh{l}_{h}")
                nc.vector.tensor_scalar_mul(dh[:], pso[HD:HD + 1, :], 1.0)
                nc.vector.tensor_scalar_mul(
                    oT[pp:pp + 64, hc, :], pso[0:HD, :], 1.0)
                rh = stat.tile([1, TPC], F32, tag="rr", name=f"rh{l}_{h}")
                nc.vector.reciprocal_approx_fast(rh[:], dh[:])
                bb = bcp.tile([128, TPC], F32, tag="bb", name=f"bb{l}_{h}")
                nc.gpsimd.partition_broadcast(bb[:], rh[:])
                nc.vector.tensor_tensor(
                    out=oT[pp:pp + 64, hc, :], in0=oT[pp:pp + 64, hc, :],
                    in1=bb[pp:pp + 64, :], op=ALU.mult)

            held = [phase1(h) for h in range(H)]
            for h in range(H):
                phase2(h, held[h])

        def wo_residual(l, oT, s1, s2, wots):
            for mp in range(KD // 2):
                po = ps_mm.tile([128, 2, TPC], F32, tag="mm",
                                name=f"po{l}_{mp}")
                for j in range(2):
                    m = 2 * mp + j
                    wot = wots[m]
                    for k in range(KD):
                        nc.tensor.matmul(po[:, j, :], wot[:, k, :],
                                         oT[:, k, :],
                                         start=(k == 0), stop=(k == KD - 1))
                    nc.vector.tensor_tensor(
                        out=xt[:, m, :], in0=xt[:, m, :].bitcast(F32),
                        in1=po[:, j, :], op=ALU.add)
                for j in range(2):
                    ln_stats_k(xt, s1, s2, 2 * mp + j)

        def ffn(l, xtl2, rs_bs2, s1, s2, xb_next):
            b1_t = bias.tile([128, M1], F32, tag="bias", name=f"b1{l}")
            nc.sync.dma_start(out=b1_t[:], in_=inp[_f("b1", l)][:])
            b2_t = bias.tile([128, M2], F32, tag="bias", name=f"b2{l}")
            nc.sync.dma_start(out=b2_t[:], in_=inp[_f("b2", l)][:])
            hT = hp.tile([128, M1, TPC], BF16, tag="h", name=f"hT{l}")
            for mp in range(M1 // 2):
                p1 = ps_mm.tile([128, 2, TPC], F32, tag="mm",
                                name=f"p1{l}_{mp}")
                for j in range(2):
                    m = 2 * mp + j
                    w1t = wq_p.tile([128, KD, 128], BF16, tag="wqk",
                                    name=f"w1{l}_{m}")
                    nc.sync.dma_start(out=w1t[:], in_=inp[_f("w1", l)][:, m])
                    for k in range(KD):
                        nc.tensor.matmul(p1[:, j, :], w1t[:, k, :],
                                         xtl2[:, k, :],
                                         start=(k == 0), stop=(k == KD - 1))
                    u = tmp.tile([128, TPC], F32, tag="tmp",
                                 name=f"u{l}_{m}")
                    nc.vector.tensor_tensor(
                        out=u[:], in0=p1[:, j, :], in1=rs_bs2[:],
                        op=ALU.mult)
                    nc.scalar.activation(hT[:, m, :], u[:], AF.Gelu,
                                         bias=b1_t[:, m:m + 1])
            for mp in range(M2 // 2):
                p2 = ps_mm.tile([128, 2, TPC], F32, tag="mm",
                                name=f"p2{l}_{mp}")
                for j in range(2):
                    m = 2 * mp + j
                    w2t = w2_p.tile([128, M1, 128], BF16, tag="w2",
                                    name=f"w2{l}_{m}")
                    nc.sync.dma_start(out=w2t[:], in_=inp[_f("w2", l)][:, m])
                    for k in range(M1):
                        nc.tensor.matmul(p2[:, j, :], w2t[:, k, :],
                                         hT[:, k, :],
                                         start=(k == 0), stop=(k == M1 - 1))
                    nc.vector.scalar_tensor_tensor(
                        out=xt[:, m, :], in0=p2[:, j, :],
                        scalar=b2_t[:, m:m + 1],
                        in1=xt[:, m, :].bitcast(F32), op0=ALU.add,
                        op1=ALU.add)
                    nc.scalar.copy(xb_next[:, m, :],
                                   xt[:, m, :].bitcast(F32))
                for j in range(2):
                    ln_stats_k(xt, s1, s2, 2 * mp + j)

        def tap_f32(name):
            if name in taps:
                nc.sync.dma_start(out=taps[name][:], in_=xt[:].bitcast(F32))

        # ---------------- main loop ----------------
        s1, s2 = ln_alloc("ln_0")
        xb = npool.tile([128, KD, TPC], BF16, tag="n", name="xb0")
        for k in range(KD):
            ln_stats_k(xt, s1, s2, k)
            nc.scalar.copy(xb[:, k, :], xt[:, k, :].bitcast(F32))
        for l in range(L):
            nks_t = bias.tile([128, KD], F32, tag="nks", name=f"nks{l}")
            nc.sync.dma_start(out=nks_t[:], in_=inp[_f("nks", l)][:])
            m, mB_s = mean_chain(s1, f"l{l}")
            rs_bs = rs_chain(s2, m, f"l{l}")
            agk = k_stage_gather(l, xb, mB_s, rs_bs, nks_t)
            xtl = xtilde(mB_s, f"l{l}")
            n1v = npool.tile([128, KD, TPC], BF16, tag="n", name=f"n1v{l}")
            for k in range(KD):
                nc.vector.tensor_tensor(
                    out=n1v[:, k, :], in0=xtl[:, k, :], in1=rs_bs[:],
                    op=ALU.mult)
            agv = v_stage_gather(l, n1v)
            q = q_proj(l, xtl, rs_bs)
            kg = kvp.tile([128, G, KCOLS], BF16, tag="kg", name=f"kg{l}")
            for rho in range(G):
                nc.sync.dma_start(out=kg[:, rho, :], in_=agk[rho])
            vg = kvp.tile([128, G, 2 * VSEG], BF16, tag="vg", name=f"vg{l}")
            for rho in range(G):
                nc.sync.dma_start(out=vg[:, rho, :], in_=agv[rho])
            oT = op.tile([128, KD, TPC], BF16, tag="oT", name=f"oT{l}")
            # prefetch wo strips on the scalar HWDGE queue during attention
            wots = []
            for mw in range(KD):
                wot = wq_p.tile([128, KD, 128], BF16, tag="wqk",
                                name=f"wo{l}_{mw}")
                nc.scalar.dma_start(out=wot[:], in_=inp[_f("wo", l)][:, mw])
                wots.append(wot)
            attention(l, q, kg, vg, oT)
            s1a, s2a = ln_alloc(f"ln2_{l}")
            wo_residual(l, oT, s1a, s2a, wots)
            if l == 0:
                tap_f32("xa_0")
            m2, m2B_s = mean_chain(s1a, f"f{l}")
            xtl2 = xtilde(m2B_s, f"f{l}")
            rs_bs2 = rs_chain(s2a, m2, f"f{l}")
            s1, s2 = ln_alloc(f"ln1_{l + 1}")
            xb = npool.tile([128, KD, TPC], BF16, tag="n", name=f"xb{l + 1}")
            ffn(l, xtl2, rs_bs2, s1, s2, xb)
            if l == 0:
                tap_f32("x_1")

        # final LN -> nf
        mf, mfB_s = mean_chain(s1, "fin")
        xtlf = xtilde(mfB_s, "fin")
        rs_bsf = rs_chain(s2, mf, "fin")
        nf = npool.tile([128, KD, TPC], BF16, tag="n", name="nf")
        for k in range(KD):
            nc.vector.tensor_tensor(
                out=nf[:, k, :], in0=xtlf[:, k, :], in1=rs_bsf[:],
                op=ALU.mult)
        if "nf" in taps:
            f = stg.tile([128, KD, TPC], F32, tag="tapf", name="tpnf")
            nc.scalar.copy(f[:], nf[:])
            nc.sync.dma_start(out=taps["nf"][:], in_=f[:])

        # final AllGather of nf, then vocab-sharded logits
        nf_in = dram.tile([128, NFCOLS], BF16, tag="nfi")
        nf_out = dram.tile([G, 128, NFCOLS], BF16, tag="nfo")
        nc.sync.dma_start(out=nf_in[:], in_=nf[:])
        nc.gpsimd.collective_compute(
            "AllGather", ALU.bypass, replica_groups=REPLICA_GROUPS,
            ins=[nf_in.opt()], outs=[nf_out.opt()])
        nfg = hp.tile([128, G, KD, TPC], BF16, tag="h", name="nfg")
        for rho in range(G):
            nc.sync.dma_start(out=nfg[:, rho], in_=nf_out[rho])

        for vb in range(NVB):
            ebt = embp.tile([128, KD, 128], BF16, tag="emb", name=f"eb{vb}")
            nc.sync.dma_start(out=ebt[:], in_=inp["emb"][:, vb])
            for half in range(2):
                pl = ps_mm.tile([128, 512], F32, tag="mm",
                                name=f"pl{vb}_{half}")
                for k in range(KD):
                    nc.tensor.matmul(pl[:], ebt[:, k, :],
                                     nfg[:, ts(half, 2), k, :],
                                     start=(k == 0), stop=(k == KD - 1))
                lo = lout.tile([128, 512], BF16, tag="lo",
                               name=f"lo{vb}_{half}")
                if (vb + half) % 2 == 0:
                    nc.scalar.copy(lo[:], pl[:])
                else:
                    nc.vector.tensor_scalar_mul(lo[:], pl[:], 1.0)
                nc.sync.dma_start(out=logits[ts(vb, 128), ts(half, 512)],
                                  in_=lo[:])


# ------------------------------------------------------------------
# Host side
# ------------------------------------------------------------------

def _kfold(w):
    """[in, out] -> [128, in//128, out]."""
    i, o = w.shape
    return np.ascontiguousarray(w.reshape(i // 128, 128, o).transpose(1, 0, 2))


def _mslice(w):
    """[in, out] -> [128, out//128, in//128, 128] contiguous strips."""
    i, o = w.shape
    t = w.reshape(i // 128, 128, o // 128, 128)
    return np.ascontiguousarray(t.transpose(1, 2, 0, 3))


def _cols(v):
    """[n] -> [128, n//128] per-partition bias columns."""
    return np.ascontiguousarray(v.reshape(-1, 128).T)


def _bf(a):
    return np.ascontiguousarray(a).astype(BF16NP)


def prep_inputs(inputs):
    f = lambda a: np.asarray(a, np.float32)
    tokens = np.asarray(inputs["tokens"])
    tok_emb, pos_emb = f(inputs["tok_emb"]), f(inputs["pos_emb"])
    ln1_g = f(inputs["ln1_g"])
    wq, wk = f(inputs["wq"]), f(inputs["wk"])
    wv, wo = f(inputs["wv"]), f(inputs["wo"])
    ln2_g, ln2_b = f(inputs["ln2_g"]), f(inputs["ln2_b"])
    w1, b1 = f(inputs["w1"]), f(inputs["b1"])
    w2, b2 = f(inputs["w2"]), f(inputs["b2"])
    lnf_g = f(inputs["lnf_g"])

    sc = 1.0 / np.sqrt(HD)
    x0 = tok_emb[tokens] + pos_emb[:S][None]          # [B, S, D]
    ones = np.ones((128, 1), np.float32)

    # shared (identical on all cores) weight tensors
    shared = {"ones": ones}
    for l in range(L):
        shared[_f("wq", l)] = _bf(_mslice(ln1_g[l][:, None] * wq[l] * sc))
        shared[_f("wk", l)] = _bf(_mslice(ln1_g[l][:, None] * wk[l]))
        shared[_f("nks", l)] = _cols(-np.asarray(
            _bf(ln1_g[l][:, None] * wk[l]), np.float32).sum(0))
        shared[_f("wv", l)] = _bf(_kfold(ln1_g[l][:, None] * wv[l]))
        shared[_f("wo", l)] = _bf(_mslice(wo[l]))
        shared[_f("w1", l)] = _bf(_mslice(ln2_g[l][:, None] * w1[l]))
        shared[_f("w2", l)] = _bf(_mslice(w2[l]))
        shared[_f("b1", l)] = _cols(b1[l] + ln2_b[l] @ w1[l])
        shared[_f("b2", l)] = _cols(b2[l])

    in_maps = []
    for core in range(N_CORES):
        g, r = core // G, core % G
        A_blk, B_blk = r, 7 - r
        m = dict(shared)
        xo = np.concatenate([x0[g, 128 * A_blk:128 * A_blk + 128],
                             x0[g, 128 * B_blk:128 * B_blk + 128]], 0)
        m["x0"] = _kfold(np.ascontiguousarray(xo.T))
        m01 = np.zeros((128, NSLOT, BLK), np.float32)
        kp = np.arange(128)[:, None]
        qf = np.arange(128)[None, :]
        for s in range(NSLOT):
            qb = A_blk if s < 4 else B_blk
            kb = s if s < 4 else s - 4
            m01[:, s, :] = (128 * kb + kp <= 128 * qb + qf)
        m["m01"] = _bf(m01)
        v0 = r * VS
        v1 = min(v0 + VS, V)
        epad = np.zeros((D, VSP), np.float32)
        epad[:, :v1 - v0] = (tok_emb[v0:v1] * lnf_g[None, :]).T
        m["emb"] = _bf(_mslice(epad))
        in_maps.append(m)
    return in_maps


_CACHED = {}


def _get_program(debug_taps=False):
    key = bool(debug_taps)
    if key not in _CACHED:
        _CACHED[key] = build_program(debug_taps)
    return _CACHED[key]


def run(inputs, debug_taps=False, trace=False, **kw):
    nc = _get_program(debug_taps)
    in_maps = prep_inputs(inputs)
    return run_bass_kernel_spmd(nc, in_maps, list(range(N_CORES)),
                                trace=trace, **kw)


# token column -> natural token index within a group's 1024 tokens
def _colperm():
    perm = np.empty(T, np.int64)
    for c in range(T):
        rho, rem = divmod(c, 256)
        half, qf = divmod(rem, 128)
        blkid = rho if half == 0 else 7 - rho
        perm[c] = 128 * blkid + qf
    return perm


def assemble(results, inputs):
    lnf_b = np.asarray(inputs["lnf_b"], np.float32)
    tok_emb = np.asarray(inputs["tok_emb"], np.float32)
    perm = _colperm()
    out = np.empty((B, S, V), np.float32)
    for b in range(B):
        for r in range(G):
            v0 = r * VS
            v1 = min(v0 + VS, V)
            part = results[b * G + r]["logits"][:v1 - v0, :]  # [rows, T]
            out[b, perm, v0:v1] = part.T.astype(np.float32)
    if np.any(lnf_b):
        out += (tok_emb @ lnf_b)[None, None, :]
    return out


def kernel(**inputs):
    res = run(inputs)
    return assemble(res.results, inputs)


if __name__ == "__main__":
    print("building program...")
    build_program()
    print("build + compile OK")
